# revision 16
# baseline (speedup 1.0000x reference)
"""Trainium2 Bass kernel for multi-head NonLocalBlock1D (B=16, C=512, T=1024, 3 heads).

Strategy:
  - Data-parallel over batch: 8 cores x 2 batches each, zero collectives.
  - Temporal dilated convs folded into the g/theta/phi 1x1 projections
    (host-side weight folding): proj(conv_h(x)) = sum_k (proj_w @ Ck) @ shift_k(x).
  - theta/phi projections + scores + gT + fx run in float32r (precision-critical
    through exp). Softmax normalization is applied to exp BEFORE the yraw matmul
    (weights in [0,1] scaled x128), which makes the yraw and W matmuls safe in
    fp8 e4m3 with perf_mode=DoubleRow (K=256 per matmul, 2x PE throughput):
      yraw = gtt8^T ydot8 (pairs over s-blocks), W = wt8^T yall8 (pairs over kc).
  - The residual x is added into the W psum via an identity matmul scaled 2^K
    (all fp8 scale factors are powers of two, folded into host-side weights,
    per-partition ACT scale tiles, and the descaled fx weights).
  - g biases ride through softmax (rows sum to 1) and are folded, with both
    BatchNorms and conv/proj bias terms, into W/fx weights + one final bias.
  - Heads processed in order [2, 0, 1] (head 2 has 1 tap -> short startup DMA),
    software-pipelined one head deep so the PE never waits on the
    exp->colsum->reciprocal->normalize chain.
"""
import numpy as np
import ml_dtypes

import concourse.bass as bass
import concourse.tile as tile
import concourse.mybir as mybir
from concourse import bacc, bass_utils
from concourse.bass_isa import ReduceOp
from contextlib import ExitStack

F32 = mybir.dt.float32
F32R = mybir.dt.float32r
F8 = mybir.dt.float8e4
BF16 = mybir.dt.bfloat16
AF = mybir.ActivationFunctionType
DR = mybir.MatmulPerfMode.DoubleRow

B, C, T, INTER, H, TL = 16, 512, 1024, 256, 3, 2
EPS = 1e-5
NCORES = 8
BPC = B // NCORES          # batches per core
XW = T + 4                 # padded x chunk width (+-2 zero pad)
HEAD_ORDER = (2, 0, 1)

_CACHE = {}


def _build():
    nc = bacc.Bacc("TRN2")
    x_d = nc.dram_tensor("x", (BPC, 128, 4, T), F32R, kind="ExternalInput")
    fw_d = nc.dram_tensor("fw", (9, 128, 3 * 4 * INTER), F32R, kind="ExternalInput")
    bias_d = nc.dram_tensor("bias", (H, 128, 4), F32, kind="ExternalInput")
    scales_d = nc.dram_tensor("scales", (128, 4), F32, kind="ExternalInput")
    wt8_d = nc.dram_tensor("wt8", (128, 6, 512), F8, kind="ExternalInput")
    i2k_d = nc.dram_tensor("i2k", (128, 128), F32R, kind="ExternalInput")
    fxT_d = nc.dram_tensor("fxT", (128, 4 * 512), F32R, kind="ExternalInput")
    cF_d = nc.dram_tensor("cF", (128, 4), F32, kind="ExternalInput")
    out_d = nc.dram_tensor("out", (BPC, C, T), F32, kind="ExternalOutput")

    with tile.TileContext(nc) as tc, ExitStack() as ctx:
        def pool(name, bufs, **kw):
            return ctx.enter_context(tc.tile_pool(name=name, bufs=bufs, **kw))

        p_const = pool("const", 1)
        p_x = pool("xp", 2)
        p_fw = pool("fwp", 4)
        p_thph = pool("thph", 2)
        p_gt = pool("gtp", 2)
        p_exp = pool("expp", 8)
        p_yd = pool("ydp", 14)
        p_yall = pool("yallp", 2)
        p_misc = pool("miscp", 2)
        p_zr = pool("zrp", 6)
        p_out = pool("outp", 2)
        p_ps = pool("ps", 8, space="PSUM")

        zz = p_const.tile([128, 2], F32, tag="zz")
        nc.vector.memset(zz[:], 0.0)
        ones_f = p_const.tile([128, 1], F32, tag="ones_f")
        nc.vector.memset(ones_f[:], 0.0078125)     # 1/128, folds x128 into recip
        ones = p_const.tile([128, 1], F32R, tag="ones")
        nc.vector.tensor_copy(ones[:], ones_f[:])

        scales = p_const.tile([128, 4], F32, tag="scales")
        wt8 = p_const.tile([128, 6, 512], F8, tag="wt8")
        i2k = p_const.tile([128, 128], F32R, tag="i2k")
        fxt = p_const.tile([128, 4 * 512], F32R, tag="fxT")
        cft = p_const.tile([128, 4], F32, tag="cF")
        biases = [p_const.tile([128, 4], F32, tag=f"bias{h}", name=f"bias{h}")
                  for h in range(H)]

        def load_const_early():
            # scalar queue: tiny consts needed by the first ACTs
            nc.scalar.dma_start(scales[:], scales_d.ap()[:])
            for h in HEAD_ORDER:
                nc.scalar.dma_start(biases[h][:], bias_d.ap()[h])

        def load_const_late():
            # W/fx consts, needed only ~halfway through batch 0
            nc.scalar.dma_start(wt8[:], wt8_d.ap()[:])
            nc.scalar.dma_start(i2k[:], i2k_d.ap()[:])
            nc.scalar.dma_start(fxt[:], fxT_d.ap()[:])
            nc.scalar.dma_start(cft[:], cF_d.ap()[:])

        # ---------- x tiles (both batches), strided window DMAs ----------
        xts = []
        for b in range(BPC):
            xt = p_x.tile([128, 4, XW], F32R, tag="x", name=f"x{b}")
            for cc in range(4):
                nc.vector.tensor_copy(xt[:, cc:cc + 1, 0:2], zz[:])
                nc.vector.tensor_copy(xt[:, cc:cc + 1, 2 + T:4 + T], zz[:])
            xts.append(xt)

        def load_x(b, windows):
            # one strided DMA per t-window covering all 4 c-chunks
            for lo, hi in windows:
                nc.sync.dma_start(xts[b][:, :, 2 + lo:2 + hi],
                                  x_d.ap()[b][:, :, lo:hi])

        def xs(b, cc, lo, width, dlt=0):
            base = 2 + lo + dlt
            return xts[b][:, cc:cc + 1, base:base + width]

        # ---------- weight loads (gpsimd queue, one head ahead) ----------
        fwt = {}

        def load_fw(b, h, fine=False):
            nk = 3 if h < TL else 1
            for pj in range(3):             # 0=theta, 1=phi, 2=g
                t_ = p_fw.tile([128, 3 * 4 * INTER], F32R, tag="fw",
                               name=f"fw{b}{h}{pj}")
                nsplit = 2 if (fine and pj == 0) else 1
                w_ = nk * 4 * INTER // nsplit
                for ki in range(nsplit):
                    nc.gpsimd.dma_start(
                        t_[:, ki * w_:(ki + 1) * w_],
                        fw_d.ap()[h * 3 + pj][:, ki * w_:(ki + 1) * w_])
                fwt[(h, pj)] = t_

        # ================= per-head phases =================
        def phase_A(b, h, hook=None):
            """theta/phi ([i,t] f32r) + gT (-> gtt8 fp8), t-half n outermost."""
            taps = [-(h + 1), 0, h + 1] if h < TL else [0]
            nk = len(taps)
            tht = p_thph.tile([128, 2 * T], F32R, tag="th", name=f"th{b}{h}")
            pht = p_thph.tile([128, 2 * T], F32R, tag="ph", name=f"ph{b}{h}")
            gtt8 = p_gt.tile([128, 8, INTER], F8, tag="gt", name=f"gt{b}{h}")
            for n in range(2):
                for pj, dst in ((0, tht), (1, pht)):
                    for it in range(2):
                        ps = p_ps.tile([128, 512], F32, tag="ps")
                        cnt = 0
                        for ki, dlt in enumerate(taps):
                            for cc in range(4):
                                lhs = fwt[(h, pj)][:, (ki * 4 + cc) * INTER + it * 128:
                                                   (ki * 4 + cc) * INTER + (it + 1) * 128]
                                nc.tensor.matmul(
                                    ps[:], lhs, xs(b, cc, n * 512, 512, dlt),
                                    start=(cnt == 0), stop=(cnt == nk * 4 - 1))
                                cnt += 1
                        nc.scalar.activation(
                            dst[:, it * T + n * 512:it * T + (n + 1) * 512], ps[:],
                            AF.Identity,
                            bias=biases[h][:, pj * 2 + it:pj * 2 + it + 1])
                for sb in range(4 * n, 4 * n + 4):
                    ps = p_ps.tile([128, 512], F32, tag="ps")
                    cnt = 0
                    for ki, dlt in enumerate(taps):
                        for cc in range(4):
                            nc.tensor.matmul(
                                ps[:, 0:INTER],
                                xs(b, cc, sb * 128, 128, dlt),
                                fwt[(h, 2)][:, (ki * 4 + cc) * INTER:(ki * 4 + cc + 1) * INTER],
                                start=(cnt == 0), stop=(cnt == nk * 4 - 1))
                            cnt += 1
                    nc.scalar.activation(gtt8[:, sb:sb + 1, :], ps[:, 0:INTER],
                                         AF.Identity, scale=scales[:, h:h + 1])
                if n == 0 and hook is not None:
                    hook()
            return tht, pht, gtt8

        def phase_B(b, h, tht, pht):
            """scores -> exp -> colsum -> recip -> normalized fp8 weights."""
            yd_all = []
            for n in range(2):
                exs = []
                tr1 = []
                for sb in range(8):
                    scp = p_ps.tile([128, 512], F32, tag="ps")
                    for ic in range(2):
                        nc.tensor.matmul(
                            scp[:],
                            pht[:, ic * T + sb * 128:ic * T + (sb + 1) * 128],
                            tht[:, ic * T + n * 512:ic * T + (n + 1) * 512],
                            start=(ic == 0), stop=(ic == 1))
                    ex = p_exp.tile([128, 512], F32R, tag="exp")
                    nc.scalar.activation(ex[:], scp[:], AF.Exp)
                    exs.append(ex)
                    if sb % 2 == 1:          # colsum add-tree, level 1
                        t = p_misc.tile([128, 512], F32R, tag="tr", bufs=6)
                        nc.vector.tensor_add(t[:], exs[sb - 1][:], ex[:])
                        tr1.append(t)
                ta = p_misc.tile([128, 512], F32R, tag="tr", bufs=6)
                nc.vector.tensor_add(ta[:], tr1[0][:], tr1[1][:])
                tb = p_misc.tile([128, 512], F32R, tag="tr", bufs=6)
                nc.vector.tensor_add(tb[:], tr1[2][:], tr1[3][:])
                cs = p_misc.tile([128, 512], F32R, tag="cs", bufs=2)
                nc.vector.tensor_add(cs[:], ta[:], tb[:])
                # cross-partition sum via one ones-matmul on the tree result
                cst = p_ps.tile([128, 512], F32, tag="ps")
                nc.tensor.matmul(cst[0:1, :], ones[:], cs[:], start=True, stop=True)
                rcs = p_misc.tile([128, 512], F32, tag="cs", bufs=2, name="rcs")
                nc.vector.reciprocal_approx_fast(rcs[0:1, :], cst[0:1, :])
                rbc = p_misc.tile([128, 512], F32, tag="rbc", bufs=2)
                nc.gpsimd.partition_broadcast(rbc[:], rcs[0:1, :])
                yds = []
                for p in range(4):
                    yd = p_yd.tile([128, 2, 512], F8, tag="yd")
                    for j in range(2):
                        nc.vector.tensor_mul(yd[:, j:j + 1, :], exs[2 * p + j][:],
                                             rbc[:])
                    yds.append(yd)
                yd_all.append(yds)
            return yd_all

        def phase_C(b, h, gtt8, yd_all, yall8):
            """yraw via fp8 DoubleRow -> yall8."""
            for n in range(2):
                yds = yd_all[n]
                for ic in range(2):
                    yr = p_ps.tile([128, 512], F32, tag="ps")
                    for p in range(4):
                        nc.tensor.matmul(
                            yr[:],
                            gtt8[:, 2 * p:2 * p + 2, ic * 128:(ic + 1) * 128],
                            yds[p][:, :, :],
                            start=(p == 0), stop=(p == 3), perf_mode=DR)
                    kc = h * 2 + ic
                    nc.scalar.activation(
                        yall8[:, kc:kc + 1, n * 512:(n + 1) * 512], yr[:],
                        AF.Identity, scale=0.0078125)

        def phase_W(b, yall8):
            """W (fp8 DoubleRow) + scaled-identity residual -> zr tiles."""
            zrs = []
            for n in range(2):
                zrt = []
                for oc in range(4):
                    ps = p_ps.tile([128, 512], F32, tag="ps")
                    nc.tensor.matmul(ps[:], i2k[:], xs(b, oc, n * 512, 512),
                                     start=True, stop=False,
                                     skip_group_check=True)
                    for q in range(3):
                        nc.tensor.matmul(
                            ps[:],
                            wt8[:, 2 * q:2 * q + 2, oc * 128:(oc + 1) * 128],
                            yall8[:, 2 * q:2 * q + 2, n * 512:(n + 1) * 512],
                            start=False, stop=(q == 2), perf_mode=DR,
                            skip_group_check=True)
                    zr = p_zr.tile([128, 512], F32R, tag="zr")
                    nc.vector.tensor_copy(zr[:], ps[:])
                    zrt.append(zr)
                zrs.append(zrt)
            return zrs

        OUT_Q = (nc.sync, nc.gpsimd, nc.scalar, nc.sync)

        def phase_fx(b, zrs):
            for n in range(2):
                zrt = zrs[n]
                for mo in range(4):
                    ps = p_ps.tile([128, 512], F32, tag="ps")
                    for kc in range(4):
                        nc.tensor.matmul(
                            ps[:],
                            fxt[:, kc * 512 + mo * 128:kc * 512 + (mo + 1) * 128],
                            zrt[kc][:],
                            start=(kc == 0), stop=(kc == 3))
                    ot = p_out.tile([128, 512], F32, tag="o")
                    nc.scalar.activation(ot[:], ps[:], AF.Identity,
                                         bias=cft[:, mo:mo + 1])
                    OUT_Q[mo].dma_start(
                        out_d.ap()[b, mo * 128:(mo + 1) * 128,
                                   n * 512:(n + 1) * 512],
                        ot[:])

        # ================= schedule =================
        # Per-queue DMA completion semaphores are monotonic: a consumer waits
        # for every earlier-emitted DMA on that queue. So emission order on
        # each queue strictly follows consumption order, and batch-1 loads
        # are deferred past batch-0's hot startup window.
        load_seq = [(b, h) for b in range(BPC) for h in HEAD_ORDER]
        load_fw(*load_seq[0], fine=True)
        load_x(0, [(0, 260), (260, 516)])
        load_const_early()
        next_load = 1

        def emit_next_load():
            nonlocal next_load
            if next_load < len(load_seq):
                load_fw(*load_seq[next_load])
                next_load += 1

        pend = None        # (b, h, gtt8, yd_all, yall8) awaiting phase_C
        done_A = {}
        for b in range(BPC):
            yall8 = p_yall.tile([128, 6, T], F8, tag="yall", name=f"yall{b}")
            for hi, h in enumerate(HEAD_ORDER):
                if (b, h) in done_A:
                    tht, pht, gtt8 = done_A.pop((b, h))
                else:
                    hook = (lambda: load_x(0, [(516, 772), (772, 1024)])) \
                        if (b == 0 and hi == 0) else None
                    tht, pht, gtt8 = phase_A(b, h, hook)
                    emit_next_load()
                if b == 0 and hi == 1:
                    load_const_late()
                if b == 0 and hi == 2:
                    load_x(1, [(0, 516), (516, 1024)])
                if pend is not None:
                    phase_C(*pend)
                    pend = None
                yd_all = phase_B(b, h, tht, pht)
                pend = (b, h, gtt8, yd_all, yall8)
            phase_C(*pend)
            pend = None
            zrs = phase_W(b, yall8)
            if b + 1 < BPC:           # cover the W->zr->fx stall with next A
                done_A[(b + 1, HEAD_ORDER[0])] = phase_A(b + 1, HEAD_ORDER[0])
                emit_next_load()
            phase_fx(b, zrs)

    nc.compile()
    return nc


def _prep(inputs):
    f = np.float32
    x = np.asarray(inputs["x"], f)
    tconv_w = np.asarray(inputs["tconv_w"], f)
    g_w = np.asarray(inputs["g_w"], f)
    g_b = np.asarray(inputs["g_b"], f)
    theta_w = np.asarray(inputs["theta_w"], f)
    theta_b = np.asarray(inputs["theta_b"], f)
    phi_w = np.asarray(inputs["phi_w"], f)
    phi_b = np.asarray(inputs["phi_b"], f)
    W_w = np.asarray(inputs["W_w"], f)
    W_b = np.asarray(inputs["W_b"], f)

    s1 = np.asarray(inputs["bn1_gamma"], f) / np.sqrt(np.asarray(inputs["bn1_var"], f) + EPS)
    s2 = np.asarray(inputs["bn2_gamma"], f) / np.sqrt(np.asarray(inputs["bn2_var"], f) + EPS)
    fx_w = np.asarray(inputs["fx_w"], f)

    # fold g biases (softmax rows sum to 1) + BN1 into W / cz
    g_ball = g_b.reshape(H * INTER)
    Wp = (W_w * s1[:, None]).astype(f)
    cz = (s1 * (W_w @ g_ball + W_b - np.asarray(inputs["bn1_mean"], f))
          + np.asarray(inputs["bn1_beta"], f)).astype(f)
    fxp = (fx_w * s2[:, None]).astype(f)
    cF = (s2 * (fx_w @ cz + np.asarray(inputs["fx_b"], f) - np.asarray(inputs["bn2_mean"], f))
          + np.asarray(inputs["bn2_beta"], f)).astype(f)

    # folded projection weights, [c, i] layout per (h, proj, tap)
    fw = np.zeros((9, 128, 3 * 4 * INTER), f)
    folds_g = []
    for h in range(H):
        for pj, pw in enumerate((theta_w, phi_w, g_w)):
            if h < TL:
                folds = [(pw[h] @ tconv_w[h, :, 0, k, :]).T for k in range(3)]
            else:
                folds = [pw[h].T]
            if pj == 2:
                folds_g.append(folds)
            arr = np.stack(folds)                      # (nk, 512, 256)
            nk = arr.shape[0]
            arr = arr.reshape(nk, 4, 128, INTER).transpose(2, 0, 1, 3)
            fw[h * 3 + pj, :, :nk * 4 * INTER] = arr.reshape(128, nk * 4 * INTER)

    # exact per-head |g| bound (|y| <= max|g| since softmax rows are convex)
    gmax = np.zeros(H, f)
    xp = np.pad(x, ((0, 0), (0, 0), (2, 2)))
    for h in range(H):
        taps = [-(h + 1), 0, h + 1] if h < TL else [0]
        gh = np.zeros((B, INTER, T), np.float32)
        for ki, dlt in enumerate(taps):
            gh += np.einsum("ci,bct->bit", folds_g[h][ki],
                            xp[:, :, 2 + dlt:2 + dlt + T].astype(np.float32))
        gmax[h] = np.abs(gh).max()

    seg = np.floor(np.log2(224.0 / gmax)).astype(np.int32)       # gtt8/yall8 scale
    # W fp8 scale: product scale 2^K uniform across head blocks
    wabs = np.array([np.abs(Wp[:, h * 2 * 128:(h + 1) * 2 * 128]).max()
                     for h in range(H)])
    K = int(np.floor(np.min(np.log2(224.0 / wabs) + seg)))
    swh = [2.0 ** (K - int(seg[h])) for h in range(H)]

    E4 = ml_dtypes.float8_e4m3
    wt8 = np.zeros((128, 6, 512), E4)
    for h in range(H):
        blk = Wp[:, h * 256:(h + 1) * 256] * swh[h]              # (512 o, 256 c)
        wt8[:, 2 * h:2 * h + 2, :] = (
            blk.T.reshape(2, 128, 512).transpose(1, 0, 2).astype(E4))

    i2k = (np.eye(128, dtype=f) * (2.0 ** K))
    fxp_d = (fxp * (2.0 ** -K)).astype(f)

    scales = np.zeros((128, 4), f)
    for h in range(H):
        scales[:, h] = 2.0 ** int(seg[h])

    bias_sb = np.stack([
        np.concatenate([theta_b[h].reshape(2, 128).T, phi_b[h].reshape(2, 128).T], axis=1)
        for h in range(H)]).astype(f)                   # (3, 128, 4)

    fxT_sb = fxp_d.T.reshape(4, 128, 512).transpose(1, 0, 2).reshape(128, 4 * 512).copy()
    cF_sb = cF.reshape(4, 128).T.copy()
    x_sb = x.reshape(B, 4, 128, T).transpose(0, 2, 1, 3)       # (B, 128, 4, T)

    common = {"fw": fw, "bias": bias_sb, "scales": scales, "wt8": wt8,
              "i2k": i2k, "fxT": fxT_sb, "cF": cF_sb}
    in_maps = []
    for c in range(NCORES):
        m = dict(common)
        m["x"] = np.ascontiguousarray(x_sb[c * BPC:(c + 1) * BPC])
        in_maps.append(m)
    return in_maps


def kernel(**inputs) -> np.ndarray:
    if "nc" not in _CACHE:
        _CACHE["nc"] = _build()
    nc = _CACHE["nc"]
    in_maps = _prep(inputs)
    res = bass_utils.run_bass_kernel_spmd(nc, in_maps, core_ids=list(range(NCORES)))
    out = np.empty((B, C, T), np.float32)
    for c in range(NCORES):
        out[c * BPC:(c + 1) * BPC] = res.results[c]["out"]
    return out


# revision 17
# speedup vs baseline: 1.3164x; 1.3164x over previous
"""Trainium2 Bass kernel for multi-head NonLocalBlock1D (B=16, C=512, T=1024, 3 heads).

Strategy:
  - Data-parallel over batch: 8 cores x 2 batches each, zero collectives.
  - Temporal dilated convs folded into the g/theta/phi 1x1 projections
    (host-side weight folding): proj(conv_h(x)) = sum_k (proj_w @ Ck) @ shift_k(x).
  - theta/phi projections + scores + gT + fx run in float32r (precision-critical
    through exp). Softmax normalization is applied to exp BEFORE the yraw matmul
    (weights in [0,1] scaled x128), which makes the yraw and W matmuls safe in
    fp8 e4m3 with perf_mode=DoubleRow (K=256 per matmul, 2x PE throughput):
      yraw = gtt8^T ydot8 (pairs over s-blocks), W = wt8^T yall8 (pairs over kc).
  - The residual x is added into the W psum via an identity matmul scaled 2^K
    (all fp8 scale factors are powers of two, folded into host-side weights,
    per-partition ACT scale tiles, and the descaled fx weights).
  - g biases ride through softmax (rows sum to 1) and are folded, with both
    BatchNorms and conv/proj bias terms, into W/fx weights + one final bias.
  - Heads processed in order [2, 0, 1] (head 2 has 1 tap -> short startup DMA),
    software-pipelined one head deep so the PE never waits on the
    exp->colsum->reciprocal->normalize chain.
"""
import numpy as np
import ml_dtypes

import concourse.bass as bass
import concourse.tile as tile
import concourse.mybir as mybir
from concourse import bacc, bass_utils
from concourse.bass_isa import ReduceOp
from contextlib import ExitStack

F32 = mybir.dt.float32
F32R = mybir.dt.float32r
F8 = mybir.dt.float8e4
BF16 = mybir.dt.bfloat16
AF = mybir.ActivationFunctionType
DR = mybir.MatmulPerfMode.DoubleRow

B, C, T, INTER, H, TL = 16, 512, 1024, 256, 3, 2
EPS = 1e-5
NCORES = 8
BPC = B // NCORES          # batches per core
XW = T + 4                 # padded x chunk width (+-2 zero pad)
HEAD_ORDER = (2, 0, 1)

_CACHE = {}


def _build():
    nc = bacc.Bacc("TRN2")
    x_d = nc.dram_tensor("x", (BPC, 128, 4, T), F32R, kind="ExternalInput")
    fw_d = nc.dram_tensor("fw", (9, 128, 3 * 4 * INTER), F32R, kind="ExternalInput")
    bias_d = nc.dram_tensor("bias", (H, 128, 4), F32, kind="ExternalInput")
    scales_d = nc.dram_tensor("scales", (128, 4), F32, kind="ExternalInput")
    wt8_d = nc.dram_tensor("wt8", (128, 6, 512), F8, kind="ExternalInput")
    i2k_d = nc.dram_tensor("i2k", (128, 128), F32R, kind="ExternalInput")
    fxT_d = nc.dram_tensor("fxT", (128, 4 * 512), F32R, kind="ExternalInput")
    cF_d = nc.dram_tensor("cF", (128, 4), F32, kind="ExternalInput")
    out_d = nc.dram_tensor("out", (BPC, C, T), F32, kind="ExternalOutput")

    with tile.TileContext(nc) as tc, ExitStack() as ctx:
        def pool(name, bufs, **kw):
            return ctx.enter_context(tc.tile_pool(name=name, bufs=bufs, **kw))

        p_const = pool("const", 1)
        p_x = pool("xp", 2)
        p_fw = pool("fwp", 4)
        p_thph = pool("thph", 2)
        p_gt = pool("gtp", 2)
        p_exp = pool("expp", 9)
        p_yd = pool("ydp", 16)
        p_yall = pool("yallp", 2)
        p_misc = pool("miscp", 2)
        p_zr = pool("zrp", 8)
        p_out = pool("outp", 2)
        p_ps = pool("ps", 8, space="PSUM")

        zz = p_const.tile([128, 2], F32, tag="zz")
        nc.vector.memset(zz[:], 0.0)
        ones_f = p_const.tile([128, 1], F32, tag="ones_f")
        nc.vector.memset(ones_f[:], 0.0078125)     # 1/128, folds x128 into recip
        ones = p_const.tile([128, 1], F32R, tag="ones")
        nc.vector.tensor_copy(ones[:], ones_f[:])

        scales = p_const.tile([128, 4], F32, tag="scales")
        wt8 = p_const.tile([128, 6, 512], F8, tag="wt8")
        i2k = p_const.tile([128, 128], F32R, tag="i2k")
        fxt = p_const.tile([128, 4 * 512], F32R, tag="fxT")
        cft = p_const.tile([128, 4], F32, tag="cF")
        biases = [p_const.tile([128, 4], F32, tag=f"bias{h}", name=f"bias{h}")
                  for h in range(H)]

        def load_const_early():
            # scalar queue: tiny consts needed by the first ACTs
            nc.scalar.dma_start(scales[:], scales_d.ap()[:])
            for h in HEAD_ORDER:
                nc.scalar.dma_start(biases[h][:], bias_d.ap()[h])

        def load_const_late():
            # W/fx consts, needed only ~halfway through batch 0
            nc.scalar.dma_start(wt8[:], wt8_d.ap()[:])
            nc.scalar.dma_start(i2k[:], i2k_d.ap()[:])
            nc.scalar.dma_start(fxt[:], fxT_d.ap()[:])
            nc.scalar.dma_start(cft[:], cF_d.ap()[:])

        # ---------- x tiles (both batches), strided window DMAs ----------
        xts = []
        for b in range(BPC):
            xt = p_x.tile([128, 4, XW], F32R, tag="x", name=f"x{b}")
            for cc in range(4):
                nc.vector.tensor_copy(xt[:, cc:cc + 1, 0:2], zz[:])
                nc.vector.tensor_copy(xt[:, cc:cc + 1, 2 + T:4 + T], zz[:])
            xts.append(xt)

        def load_x(b, windows):
            # one strided DMA per t-window covering all 4 c-chunks
            for lo, hi in windows:
                nc.sync.dma_start(xts[b][:, :, 2 + lo:2 + hi],
                                  x_d.ap()[b][:, :, lo:hi])

        def xs(b, cc, lo, width, dlt=0):
            base = 2 + lo + dlt
            return xts[b][:, cc:cc + 1, base:base + width]

        # ---------- weight loads (gpsimd queue, one head ahead) ----------
        fwt = {}

        def load_fw(b, h, fine=False):
            nk = 3 if h < TL else 1
            for pj in range(3):             # 0=theta, 1=phi, 2=g
                t_ = p_fw.tile([128, 3 * 4 * INTER], F32R, tag="fw",
                               name=f"fw{b}{h}{pj}")
                nsplit = 2 if (fine and pj == 0) else 1
                w_ = nk * 4 * INTER // nsplit
                for ki in range(nsplit):
                    nc.gpsimd.dma_start(
                        t_[:, ki * w_:(ki + 1) * w_],
                        fw_d.ap()[h * 3 + pj][:, ki * w_:(ki + 1) * w_])
                fwt[(h, pj)] = t_

        # ================= per-head phases =================
        def phase_A(b, h, hook=None):
            """theta/phi ([i,t] f32r) + gT (-> gtt8 fp8), t-half n outermost."""
            taps = [-(h + 1), 0, h + 1] if h < TL else [0]
            nk = len(taps)
            tht = p_thph.tile([128, 2 * T], F32R, tag="th", name=f"th{b}{h}")
            pht = p_thph.tile([128, 2 * T], F32R, tag="ph", name=f"ph{b}{h}")
            gtt8 = p_gt.tile([128, 8, INTER], F8, tag="gt", name=f"gt{b}{h}")
            for n in range(2):
                for pj, dst in ((0, tht), (1, pht)):
                    for it in range(2):
                        ps = p_ps.tile([128, 512], F32, tag="ps")
                        cnt = 0
                        for ki, dlt in enumerate(taps):
                            for cc in range(4):
                                lhs = fwt[(h, pj)][:, (ki * 4 + cc) * INTER + it * 128:
                                                   (ki * 4 + cc) * INTER + (it + 1) * 128]
                                nc.tensor.matmul(
                                    ps[:], lhs, xs(b, cc, n * 512, 512, dlt),
                                    start=(cnt == 0), stop=(cnt == nk * 4 - 1))
                                cnt += 1
                        nc.scalar.activation(
                            dst[:, it * T + n * 512:it * T + (n + 1) * 512], ps[:],
                            AF.Identity,
                            bias=biases[h][:, pj * 2 + it:pj * 2 + it + 1])
                for sb in range(4 * n, 4 * n + 4):
                    ps = p_ps.tile([128, 512], F32, tag="ps")
                    cnt = 0
                    for ki, dlt in enumerate(taps):
                        for cc in range(4):
                            nc.tensor.matmul(
                                ps[:, 0:INTER],
                                xs(b, cc, sb * 128, 128, dlt),
                                fwt[(h, 2)][:, (ki * 4 + cc) * INTER:(ki * 4 + cc + 1) * INTER],
                                start=(cnt == 0), stop=(cnt == nk * 4 - 1))
                            cnt += 1
                    nc.scalar.activation(gtt8[:, sb:sb + 1, :], ps[:, 0:INTER],
                                         AF.Identity, scale=scales[:, h:h + 1])
                if n == 0 and hook is not None:
                    hook()
            return tht, pht, gtt8

        def phase_B(b, h, tht, pht):
            """scores -> exp -> colsum -> recip -> normalized fp8 weights."""
            yd_all = []
            for n in range(2):
                cst = p_ps.tile([128, 512], F32, tag="ps")
                exs = []
                for sb in range(8):
                    scp = p_ps.tile([128, 512], F32, tag="ps")
                    for ic in range(2):
                        nc.tensor.matmul(
                            scp[:],
                            pht[:, ic * T + sb * 128:ic * T + (sb + 1) * 128],
                            tht[:, ic * T + n * 512:ic * T + (n + 1) * 512],
                            start=(ic == 0), stop=(ic == 1))
                    ex = p_exp.tile([128, 512], F32R, tag="exp")
                    nc.scalar.activation(ex[:], scp[:], AF.Exp)
                    exs.append(ex)
                    nc.tensor.matmul(cst[0:1, :], ones[:], ex[:],
                                     start=(sb == 0), stop=(sb == 7))
                rcs = p_misc.tile([128, 512], F32, tag="cs", bufs=2, name="rcs")
                nc.vector.reciprocal_approx_fast(rcs[0:1, :], cst[0:1, :])
                rbc = p_misc.tile([128, 512], F32, tag="rbc", bufs=2)
                nc.gpsimd.partition_broadcast(rbc[:], rcs[0:1, :])
                yds = []
                for p in range(4):
                    yd = p_yd.tile([128, 2, 512], F8, tag="yd")
                    for j in range(2):
                        nc.vector.tensor_mul(yd[:, j:j + 1, :], exs[2 * p + j][:],
                                             rbc[:])
                    yds.append(yd)
                yd_all.append(yds)
            return yd_all

        def phase_C(b, h, gtt8, yd_all, yall8):
            """yraw via fp8 DoubleRow -> yall8."""
            for n in range(2):
                yds = yd_all[n]
                for ic in range(2):
                    yr = p_ps.tile([128, 512], F32, tag="ps")
                    for p in range(4):
                        nc.tensor.matmul(
                            yr[:],
                            gtt8[:, 2 * p:2 * p + 2, ic * 128:(ic + 1) * 128],
                            yds[p][:, :, :],
                            start=(p == 0), stop=(p == 3), perf_mode=DR)
                    kc = h * 2 + ic
                    nc.scalar.activation(
                        yall8[:, kc:kc + 1, n * 512:(n + 1) * 512], yr[:],
                        AF.Identity, scale=0.0078125)

        def phase_W(b, yall8):
            """W (fp8 DoubleRow) + scaled-identity residual -> zr tiles."""
            zrs = []
            for n in range(2):
                zrt = []
                for oc in range(4):
                    ps = p_ps.tile([128, 512], F32, tag="ps")
                    nc.tensor.matmul(ps[:], i2k[:], xs(b, oc, n * 512, 512),
                                     start=True, stop=False,
                                     skip_group_check=True)
                    for q in range(3):
                        nc.tensor.matmul(
                            ps[:],
                            wt8[:, 2 * q:2 * q + 2, oc * 128:(oc + 1) * 128],
                            yall8[:, 2 * q:2 * q + 2, n * 512:(n + 1) * 512],
                            start=False, stop=(q == 2), perf_mode=DR,
                            skip_group_check=True)
                    zr = p_zr.tile([128, 512], F32R, tag="zr")
                    nc.vector.tensor_copy(zr[:], ps[:])
                    zrt.append(zr)
                zrs.append(zrt)
            return zrs

        OUT_Q = (nc.sync, nc.gpsimd, nc.scalar, nc.sync)

        def phase_fx(b, zrs):
            for n in range(2):
                zrt = zrs[n]
                for mo in range(4):
                    ps = p_ps.tile([128, 512], F32, tag="ps")
                    for kc in range(4):
                        nc.tensor.matmul(
                            ps[:],
                            fxt[:, kc * 512 + mo * 128:kc * 512 + (mo + 1) * 128],
                            zrt[kc][:],
                            start=(kc == 0), stop=(kc == 3))
                    ot = p_out.tile([128, 512], F32, tag="o")
                    nc.scalar.activation(ot[:], ps[:], AF.Identity,
                                         bias=cft[:, mo:mo + 1])
                    OUT_Q[mo].dma_start(
                        out_d.ap()[b, mo * 128:(mo + 1) * 128,
                                   n * 512:(n + 1) * 512],
                        ot[:])

        # ================= schedule =================
        # Per-queue DMA completion semaphores are monotonic: a consumer waits
        # for every earlier-emitted DMA on that queue. So emission order on
        # each queue strictly follows consumption order, and batch-1 loads
        # are deferred past batch-0's hot startup window.
        load_seq = [(b, h) for b in range(BPC) for h in HEAD_ORDER]
        load_fw(*load_seq[0], fine=True)
        load_x(0, [(0, 260), (260, 516)])
        load_const_early()
        next_load = 1

        def emit_next_load():
            nonlocal next_load
            if next_load < len(load_seq):
                load_fw(*load_seq[next_load])
                next_load += 1

        pend = None        # (b, h, gtt8, yd_all, yall8) awaiting phase_C
        done_A = {}
        for b in range(BPC):
            yall8 = p_yall.tile([128, 6, T], F8, tag="yall", name=f"yall{b}")
            for hi, h in enumerate(HEAD_ORDER):
                if (b, h) in done_A:
                    tht, pht, gtt8 = done_A.pop((b, h))
                else:
                    hook = (lambda: load_x(0, [(516, 772), (772, 1024)])) \
                        if (b == 0 and hi == 0) else None
                    tht, pht, gtt8 = phase_A(b, h, hook)
                    emit_next_load()
                if b == 0 and hi == 1:
                    load_const_late()
                if b == 0 and hi == 2:
                    load_x(1, [(0, 516), (516, 1024)])
                if pend is not None:
                    phase_C(*pend)
                    pend = None
                yd_all = phase_B(b, h, tht, pht)
                pend = (b, h, gtt8, yd_all, yall8)
            phase_C(*pend)
            pend = None
            zrs = phase_W(b, yall8)
            if b + 1 < BPC:           # cover the W->zr->fx stall with next A
                done_A[(b + 1, HEAD_ORDER[0])] = phase_A(b + 1, HEAD_ORDER[0])
                emit_next_load()
            phase_fx(b, zrs)

    nc.compile()
    return nc


def _prep(inputs):
    f = np.float32
    x = np.asarray(inputs["x"], f)
    tconv_w = np.asarray(inputs["tconv_w"], f)
    g_w = np.asarray(inputs["g_w"], f)
    g_b = np.asarray(inputs["g_b"], f)
    theta_w = np.asarray(inputs["theta_w"], f)
    theta_b = np.asarray(inputs["theta_b"], f)
    phi_w = np.asarray(inputs["phi_w"], f)
    phi_b = np.asarray(inputs["phi_b"], f)
    W_w = np.asarray(inputs["W_w"], f)
    W_b = np.asarray(inputs["W_b"], f)

    s1 = np.asarray(inputs["bn1_gamma"], f) / np.sqrt(np.asarray(inputs["bn1_var"], f) + EPS)
    s2 = np.asarray(inputs["bn2_gamma"], f) / np.sqrt(np.asarray(inputs["bn2_var"], f) + EPS)
    fx_w = np.asarray(inputs["fx_w"], f)

    # fold g biases (softmax rows sum to 1) + BN1 into W / cz
    g_ball = g_b.reshape(H * INTER)
    Wp = (W_w * s1[:, None]).astype(f)
    cz = (s1 * (W_w @ g_ball + W_b - np.asarray(inputs["bn1_mean"], f))
          + np.asarray(inputs["bn1_beta"], f)).astype(f)
    fxp = (fx_w * s2[:, None]).astype(f)
    cF = (s2 * (fx_w @ cz + np.asarray(inputs["fx_b"], f) - np.asarray(inputs["bn2_mean"], f))
          + np.asarray(inputs["bn2_beta"], f)).astype(f)

    # folded projection weights, [c, i] layout per (h, proj, tap)
    fw = np.zeros((9, 128, 3 * 4 * INTER), f)
    folds_g = []
    for h in range(H):
        for pj, pw in enumerate((theta_w, phi_w, g_w)):
            if h < TL:
                folds = [(pw[h] @ tconv_w[h, :, 0, k, :]).T for k in range(3)]
            else:
                folds = [pw[h].T]
            if pj == 2:
                folds_g.append(folds)
            arr = np.stack(folds)                      # (nk, 512, 256)
            nk = arr.shape[0]
            arr = arr.reshape(nk, 4, 128, INTER).transpose(2, 0, 1, 3)
            fw[h * 3 + pj, :, :nk * 4 * INTER] = arr.reshape(128, nk * 4 * INTER)

    # exact per-head |g| bound (|y| <= max|g| since softmax rows are convex)
    gmax = np.zeros(H, f)
    xp = np.pad(x, ((0, 0), (0, 0), (2, 2)))
    for h in range(H):
        taps = [-(h + 1), 0, h + 1] if h < TL else [0]
        gh = np.zeros((B, INTER, T), np.float32)
        for ki, dlt in enumerate(taps):
            gh += np.einsum("ci,bct->bit", folds_g[h][ki],
                            xp[:, :, 2 + dlt:2 + dlt + T].astype(np.float32))
        gmax[h] = np.abs(gh).max()

    seg = np.floor(np.log2(224.0 / gmax)).astype(np.int32)       # gtt8/yall8 scale
    # W fp8 scale: product scale 2^K uniform across head blocks
    wabs = np.array([np.abs(Wp[:, h * 2 * 128:(h + 1) * 2 * 128]).max()
                     for h in range(H)])
    K = int(np.floor(np.min(np.log2(224.0 / wabs) + seg)))
    swh = [2.0 ** (K - int(seg[h])) for h in range(H)]

    E4 = ml_dtypes.float8_e4m3
    wt8 = np.zeros((128, 6, 512), E4)
    for h in range(H):
        blk = Wp[:, h * 256:(h + 1) * 256] * swh[h]              # (512 o, 256 c)
        wt8[:, 2 * h:2 * h + 2, :] = (
            blk.T.reshape(2, 128, 512).transpose(1, 0, 2).astype(E4))

    i2k = (np.eye(128, dtype=f) * (2.0 ** K))
    fxp_d = (fxp * (2.0 ** -K)).astype(f)

    scales = np.zeros((128, 4), f)
    for h in range(H):
        scales[:, h] = 2.0 ** int(seg[h])

    bias_sb = np.stack([
        np.concatenate([theta_b[h].reshape(2, 128).T, phi_b[h].reshape(2, 128).T], axis=1)
        for h in range(H)]).astype(f)                   # (3, 128, 4)

    fxT_sb = fxp_d.T.reshape(4, 128, 512).transpose(1, 0, 2).reshape(128, 4 * 512).copy()
    cF_sb = cF.reshape(4, 128).T.copy()
    x_sb = x.reshape(B, 4, 128, T).transpose(0, 2, 1, 3)       # (B, 128, 4, T)

    common = {"fw": fw, "bias": bias_sb, "scales": scales, "wt8": wt8,
              "i2k": i2k, "fxT": fxT_sb, "cF": cF_sb}
    in_maps = []
    for c in range(NCORES):
        m = dict(common)
        m["x"] = np.ascontiguousarray(x_sb[c * BPC:(c + 1) * BPC])
        in_maps.append(m)
    return in_maps


def kernel(**inputs) -> np.ndarray:
    if "nc" not in _CACHE:
        _CACHE["nc"] = _build()
    nc = _CACHE["nc"]
    in_maps = _prep(inputs)
    res = bass_utils.run_bass_kernel_spmd(nc, in_maps, core_ids=list(range(NCORES)))
    out = np.empty((B, C, T), np.float32)
    for c in range(NCORES):
        out[c * BPC:(c + 1) * BPC] = res.results[c]["out"]
    return out


# revision 18
# speedup vs baseline: 1.4266x; 1.0838x over previous
"""Trainium2 Bass kernel for multi-head NonLocalBlock1D (B=16, C=512, T=1024, 3 heads).

Strategy:
  - Data-parallel over batch: 8 cores x 2 batches each, zero collectives.
  - Temporal dilated convs folded into the g/theta/phi 1x1 projections
    (host-side weight folding): proj(conv_h(x)) = sum_k (proj_w @ Ck) @ shift_k(x).
  - theta/phi projections + scores + gT + fx run in float32r (precision-critical
    through exp). Softmax normalization is applied to exp BEFORE the yraw matmul
    (weights in [0,1] scaled x128), which makes the yraw and W matmuls safe in
    fp8 e4m3 with perf_mode=DoubleRow (K=256 per matmul, 2x PE throughput):
      yraw = gtt8^T ydot8 (pairs over s-blocks), W = wt8^T yall8 (pairs over kc).
  - The residual x is added into the W psum via an identity matmul scaled 2^K
    (all fp8 scale factors are powers of two, folded into host-side weights,
    per-partition ACT scale tiles, and the descaled fx weights).
  - g biases ride through softmax (rows sum to 1) and are folded, with both
    BatchNorms and conv/proj bias terms, into W/fx weights + one final bias.
  - Heads processed in order [2, 0, 1] (head 2 has 1 tap -> short startup DMA),
    software-pipelined one head deep so the PE never waits on the
    exp->colsum->reciprocal->normalize chain.
"""
import numpy as np
import ml_dtypes

import concourse.bass as bass
import concourse.tile as tile
import concourse.mybir as mybir
from concourse import bacc, bass_utils
from concourse.bass_isa import ReduceOp
from contextlib import ExitStack

F32 = mybir.dt.float32
F32R = mybir.dt.float32r
F8 = mybir.dt.float8e4
BF16 = mybir.dt.bfloat16
AF = mybir.ActivationFunctionType
DR = mybir.MatmulPerfMode.DoubleRow

B, C, T, INTER, H, TL = 16, 512, 1024, 256, 3, 2
EPS = 1e-5
NCORES = 8
BPC = B // NCORES          # batches per core
XW = T + 4                 # padded x chunk width (+-2 zero pad)
HEAD_ORDER = (2, 0, 1)

_CACHE = {}


def _build():
    nc = bacc.Bacc("TRN2")
    x_d = nc.dram_tensor("x", (BPC, 128, 4, T), F32R, kind="ExternalInput")
    x16_d = nc.dram_tensor("x16", (BPC, 128, 4, T), BF16, kind="ExternalInput")
    fw_d = nc.dram_tensor("fw", (9, 128, 3 * 4 * INTER), BF16, kind="ExternalInput")
    bias_d = nc.dram_tensor("bias", (H, 128, 4), F32, kind="ExternalInput")
    scales_d = nc.dram_tensor("scales", (128, 4), F32, kind="ExternalInput")
    wt8_d = nc.dram_tensor("wt8", (128, 6, 512), F8, kind="ExternalInput")
    i2k_d = nc.dram_tensor("i2k", (128, 128), F32R, kind="ExternalInput")
    fxT_d = nc.dram_tensor("fxT", (128, 4 * 512), F32R, kind="ExternalInput")
    cF_d = nc.dram_tensor("cF", (128, 4), F32, kind="ExternalInput")
    out_d = nc.dram_tensor("out", (BPC, C, T), F32, kind="ExternalOutput")

    with tile.TileContext(nc) as tc, ExitStack() as ctx:
        def pool(name, bufs, **kw):
            return ctx.enter_context(tc.tile_pool(name=name, bufs=bufs, **kw))

        p_const = pool("const", 1)
        p_x = pool("xp", 2)
        p_fw = pool("fwp", 4)
        p_thph = pool("thph", 2)
        p_gt = pool("gtp", 2)
        p_exp = pool("expp", 9)
        p_yd = pool("ydp", 16)
        p_yall = pool("yallp", 2)
        p_misc = pool("miscp", 2)
        p_zr = pool("zrp", 8)
        p_out = pool("outp", 2)
        p_ps = pool("ps", 8, space="PSUM")

        zz = p_const.tile([128, 2], F32, tag="zz")
        nc.vector.memset(zz[:], 0.0)
        ones_f = p_const.tile([128, 1], F32, tag="ones_f")
        nc.vector.memset(ones_f[:], 0.0078125)     # 1/128, folds x128 into recip
        ones = p_const.tile([128, 1], F32R, tag="ones")
        nc.vector.tensor_copy(ones[:], ones_f[:])

        scales = p_const.tile([128, 4], F32, tag="scales")
        wt8 = p_const.tile([128, 6, 512], F8, tag="wt8")
        i2k = p_const.tile([128, 128], F32R, tag="i2k")
        fxt = p_const.tile([128, 4 * 512], F32R, tag="fxT")
        cft = p_const.tile([128, 4], F32, tag="cF")
        biases = [p_const.tile([128, 4], F32, tag=f"bias{h}", name=f"bias{h}")
                  for h in range(H)]

        def load_const_early():
            # scalar queue: tiny consts needed by the first ACTs
            nc.scalar.dma_start(scales[:], scales_d.ap()[:])
            for h in HEAD_ORDER:
                nc.scalar.dma_start(biases[h][:], bias_d.ap()[h])

        def load_const_late():
            # W/fx consts, needed only ~halfway through batch 0
            nc.scalar.dma_start(wt8[:], wt8_d.ap()[:])
            nc.scalar.dma_start(i2k[:], i2k_d.ap()[:])
            nc.scalar.dma_start(fxt[:], fxT_d.ap()[:])
            nc.scalar.dma_start(cft[:], cF_d.ap()[:])

        # ---------- x tiles (both batches), strided window DMAs ----------
        xts = []
        xfs = []
        for b in range(BPC):
            xt = p_x.tile([128, 4, XW], BF16, tag="x", name=f"x{b}")
            for cc in range(4):
                nc.vector.tensor_copy(xt[:, cc:cc + 1, 0:2], zz[:])
                nc.vector.tensor_copy(xt[:, cc:cc + 1, 2 + T:4 + T], zz[:])
            xts.append(xt)
            xf = p_x.tile([128, 4, T], F32R, tag="xf", name=f"xf{b}")
            xfs.append(xf)

        def load_x(b, windows):
            # one strided DMA per t-window covering all 4 c-chunks
            for lo, hi in windows:
                nc.sync.dma_start(xts[b][:, :, 2 + lo:2 + hi],
                                  x16_d.ap()[b][:, :, lo:hi])

        def load_xf(b):
            # full-precision x for the residual path, needed only at phase_W
            nc.sync.dma_start(xfs[b][:], x_d.ap()[b][:])

        def xs(b, cc, lo, width, dlt=0):
            base = 2 + lo + dlt
            return xts[b][:, cc:cc + 1, base:base + width]

        def xsf(b, cc, lo, width):
            return xfs[b][:, cc:cc + 1, lo:lo + width]

        # ---------- weight loads (gpsimd queue, one head ahead) ----------
        fwt = {}

        def load_fw(b, h, fine=False):
            nk = 3 if h < TL else 1
            for pj in range(3):             # 0=theta, 1=phi, 2=g
                t_ = p_fw.tile([128, 3 * 4 * INTER], BF16, tag="fw",
                               name=f"fw{b}{h}{pj}")
                nsplit = 2 if (fine and pj == 0) else 1
                w_ = nk * 4 * INTER // nsplit
                for ki in range(nsplit):
                    nc.gpsimd.dma_start(
                        t_[:, ki * w_:(ki + 1) * w_],
                        fw_d.ap()[h * 3 + pj][:, ki * w_:(ki + 1) * w_])
                fwt[(h, pj)] = t_

        # ================= per-head phases =================
        def phase_A(b, h, hook=None):
            """theta/phi ([i,t] f32r) + gT (-> gtt8 fp8), t-half n outermost."""
            taps = [-(h + 1), 0, h + 1] if h < TL else [0]
            nk = len(taps)
            tht = p_thph.tile([128, 2 * T], F32R, tag="th", name=f"th{b}{h}")
            pht = p_thph.tile([128, 2 * T], F32R, tag="ph", name=f"ph{b}{h}")
            gtt8 = p_gt.tile([128, 8, INTER], F8, tag="gt", name=f"gt{b}{h}")
            for n in range(2):
                for pj, dst in ((0, tht), (1, pht)):
                    for it in range(2):
                        ps = p_ps.tile([128, 512], F32, tag="ps")
                        cnt = 0
                        for ki, dlt in enumerate(taps):
                            for cc in range(4):
                                lhs = fwt[(h, pj)][:, (ki * 4 + cc) * INTER + it * 128:
                                                   (ki * 4 + cc) * INTER + (it + 1) * 128]
                                nc.tensor.matmul(
                                    ps[:], lhs, xs(b, cc, n * 512, 512, dlt),
                                    start=(cnt == 0), stop=(cnt == nk * 4 - 1))
                                cnt += 1
                        nc.scalar.activation(
                            dst[:, it * T + n * 512:it * T + (n + 1) * 512], ps[:],
                            AF.Identity,
                            bias=biases[h][:, pj * 2 + it:pj * 2 + it + 1])
                for sb in range(4 * n, 4 * n + 4):
                    ps = p_ps.tile([128, 512], F32, tag="ps")
                    cnt = 0
                    for ki, dlt in enumerate(taps):
                        for cc in range(4):
                            nc.tensor.matmul(
                                ps[:, 0:INTER],
                                xs(b, cc, sb * 128, 128, dlt),
                                fwt[(h, 2)][:, (ki * 4 + cc) * INTER:(ki * 4 + cc + 1) * INTER],
                                start=(cnt == 0), stop=(cnt == nk * 4 - 1))
                            cnt += 1
                    nc.scalar.activation(gtt8[:, sb:sb + 1, :], ps[:, 0:INTER],
                                         AF.Identity, scale=scales[:, h:h + 1])
                if n == 0 and hook is not None:
                    hook()
            return tht, pht, gtt8

        def phase_B(b, h, tht, pht):
            """scores -> exp -> colsum -> recip -> normalized fp8 weights."""
            yd_all = []
            for n in range(2):
                cst = p_ps.tile([128, 512], F32, tag="ps")
                exs = []
                for sb in range(8):
                    scp = p_ps.tile([128, 512], F32, tag="ps")
                    for ic in range(2):
                        nc.tensor.matmul(
                            scp[:],
                            pht[:, ic * T + sb * 128:ic * T + (sb + 1) * 128],
                            tht[:, ic * T + n * 512:ic * T + (n + 1) * 512],
                            start=(ic == 0), stop=(ic == 1))
                    ex = p_exp.tile([128, 512], F32R, tag="exp")
                    nc.scalar.activation(ex[:], scp[:], AF.Exp)
                    exs.append(ex)
                    nc.tensor.matmul(cst[0:1, :], ones[:], ex[:],
                                     start=(sb == 0), stop=(sb == 7))
                rcs = p_misc.tile([128, 512], F32, tag="cs", bufs=2, name="rcs")
                nc.vector.reciprocal_approx_fast(rcs[0:1, :], cst[0:1, :])
                rbc = p_misc.tile([128, 512], F32, tag="rbc", bufs=2)
                nc.gpsimd.partition_broadcast(rbc[:], rcs[0:1, :])
                yds = []
                for p in range(4):
                    yd = p_yd.tile([128, 2, 512], F8, tag="yd")
                    for j in range(2):
                        nc.vector.tensor_mul(yd[:, j:j + 1, :], exs[2 * p + j][:],
                                             rbc[:])
                    yds.append(yd)
                yd_all.append(yds)
            return yd_all

        def phase_C(b, h, gtt8, yd_all, yall8):
            """yraw via fp8 DoubleRow -> yall8."""
            for n in range(2):
                yds = yd_all[n]
                for ic in range(2):
                    yr = p_ps.tile([128, 512], F32, tag="ps")
                    for p in range(4):
                        nc.tensor.matmul(
                            yr[:],
                            gtt8[:, 2 * p:2 * p + 2, ic * 128:(ic + 1) * 128],
                            yds[p][:, :, :],
                            start=(p == 0), stop=(p == 3), perf_mode=DR)
                    kc = h * 2 + ic
                    nc.scalar.activation(
                        yall8[:, kc:kc + 1, n * 512:(n + 1) * 512], yr[:],
                        AF.Identity, scale=0.0078125)

        def phase_W(b, yall8):
            """W (fp8 DoubleRow) + scaled-identity residual -> zr tiles."""
            zrs = []
            for n in range(2):
                zrt = []
                for oc in range(4):
                    ps = p_ps.tile([128, 512], F32, tag="ps")
                    nc.tensor.matmul(ps[:], i2k[:], xsf(b, oc, n * 512, 512),
                                     start=True, stop=False,
                                     skip_group_check=True)
                    for q in range(3):
                        nc.tensor.matmul(
                            ps[:],
                            wt8[:, 2 * q:2 * q + 2, oc * 128:(oc + 1) * 128],
                            yall8[:, 2 * q:2 * q + 2, n * 512:(n + 1) * 512],
                            start=False, stop=(q == 2), perf_mode=DR,
                            skip_group_check=True)
                    zr = p_zr.tile([128, 512], F32R, tag="zr")
                    nc.vector.tensor_copy(zr[:], ps[:])
                    zrt.append(zr)
                zrs.append(zrt)
            return zrs

        OUT_Q = (nc.sync, nc.gpsimd, nc.scalar, nc.sync)

        def phase_fx(b, zrs):
            for n in range(2):
                zrt = zrs[n]
                for mo in range(4):
                    ps = p_ps.tile([128, 512], F32, tag="ps")
                    for kc in range(4):
                        nc.tensor.matmul(
                            ps[:],
                            fxt[:, kc * 512 + mo * 128:kc * 512 + (mo + 1) * 128],
                            zrt[kc][:],
                            start=(kc == 0), stop=(kc == 3))
                    ot = p_out.tile([128, 512], F32, tag="o")
                    nc.scalar.activation(ot[:], ps[:], AF.Identity,
                                         bias=cft[:, mo:mo + 1])
                    OUT_Q[mo].dma_start(
                        out_d.ap()[b, mo * 128:(mo + 1) * 128,
                                   n * 512:(n + 1) * 512],
                        ot[:])

        # ================= schedule =================
        # Per-queue DMA completion semaphores are monotonic: a consumer waits
        # for every earlier-emitted DMA on that queue. So emission order on
        # each queue strictly follows consumption order, and batch-1 loads
        # are deferred past batch-0's hot startup window.
        load_seq = [(b, h) for b in range(BPC) for h in HEAD_ORDER]
        load_fw(*load_seq[0], fine=True)
        load_x(0, [(0, 260), (260, 516)])
        load_const_early()
        next_load = 1

        def emit_next_load():
            nonlocal next_load
            if next_load < len(load_seq):
                load_fw(*load_seq[next_load])
                next_load += 1

        pend = None        # (b, h, gtt8, yd_all, yall8) awaiting phase_C
        done_A = {}
        for b in range(BPC):
            yall8 = p_yall.tile([128, 6, T], F8, tag="yall", name=f"yall{b}")
            for hi, h in enumerate(HEAD_ORDER):
                if (b, h) in done_A:
                    tht, pht, gtt8 = done_A.pop((b, h))
                else:
                    hook = (lambda: load_x(0, [(516, 772), (772, 1024)])) \
                        if (b == 0 and hi == 0) else None
                    tht, pht, gtt8 = phase_A(b, h, hook)
                    emit_next_load()
                if hi == 1:
                    load_xf(b)
                if b == 0 and hi == 1:
                    load_const_late()
                if b == 0 and hi == 2:
                    load_x(1, [(0, 516), (516, 1024)])
                if pend is not None:
                    phase_C(*pend)
                    pend = None
                yd_all = phase_B(b, h, tht, pht)
                pend = (b, h, gtt8, yd_all, yall8)
            phase_C(*pend)
            pend = None
            zrs = phase_W(b, yall8)
            if b + 1 < BPC:           # cover the W->zr->fx stall with next A
                done_A[(b + 1, HEAD_ORDER[0])] = phase_A(b + 1, HEAD_ORDER[0])
                emit_next_load()
            phase_fx(b, zrs)

    nc.compile()
    return nc


def _prep(inputs):
    f = np.float32
    x = np.asarray(inputs["x"], f)
    tconv_w = np.asarray(inputs["tconv_w"], f)
    g_w = np.asarray(inputs["g_w"], f)
    g_b = np.asarray(inputs["g_b"], f)
    theta_w = np.asarray(inputs["theta_w"], f)
    theta_b = np.asarray(inputs["theta_b"], f)
    phi_w = np.asarray(inputs["phi_w"], f)
    phi_b = np.asarray(inputs["phi_b"], f)
    W_w = np.asarray(inputs["W_w"], f)
    W_b = np.asarray(inputs["W_b"], f)

    s1 = np.asarray(inputs["bn1_gamma"], f) / np.sqrt(np.asarray(inputs["bn1_var"], f) + EPS)
    s2 = np.asarray(inputs["bn2_gamma"], f) / np.sqrt(np.asarray(inputs["bn2_var"], f) + EPS)
    fx_w = np.asarray(inputs["fx_w"], f)

    # fold g biases (softmax rows sum to 1) + BN1 into W / cz
    g_ball = g_b.reshape(H * INTER)
    Wp = (W_w * s1[:, None]).astype(f)
    cz = (s1 * (W_w @ g_ball + W_b - np.asarray(inputs["bn1_mean"], f))
          + np.asarray(inputs["bn1_beta"], f)).astype(f)
    fxp = (fx_w * s2[:, None]).astype(f)
    cF = (s2 * (fx_w @ cz + np.asarray(inputs["fx_b"], f) - np.asarray(inputs["bn2_mean"], f))
          + np.asarray(inputs["bn2_beta"], f)).astype(f)

    # folded projection weights (bf16), [c, i] layout per (h, proj, tap)
    fw = np.zeros((9, 128, 3 * 4 * INTER), ml_dtypes.bfloat16)
    folds_g = []
    for h in range(H):
        for pj, pw in enumerate((theta_w, phi_w, g_w)):
            if h < TL:
                folds = [(pw[h] @ tconv_w[h, :, 0, k, :]).T for k in range(3)]
            else:
                folds = [pw[h].T]
            if pj == 2:
                folds_g.append(folds)
            arr = np.stack(folds)                      # (nk, 512, 256)
            nk = arr.shape[0]
            arr = arr.reshape(nk, 4, 128, INTER).transpose(2, 0, 1, 3)
            fw[h * 3 + pj, :, :nk * 4 * INTER] = arr.reshape(128, nk * 4 * INTER)

    # exact per-head |g| bound (|y| <= max|g| since softmax rows are convex)
    gmax = np.zeros(H, f)
    xp = np.pad(x, ((0, 0), (0, 0), (2, 2)))
    for h in range(H):
        taps = [-(h + 1), 0, h + 1] if h < TL else [0]
        gh = np.zeros((B, INTER, T), np.float32)
        for ki, dlt in enumerate(taps):
            gh += np.einsum("ci,bct->bit", folds_g[h][ki],
                            xp[:, :, 2 + dlt:2 + dlt + T].astype(np.float32))
        gmax[h] = np.abs(gh).max()

    seg = np.floor(np.log2(224.0 / gmax)).astype(np.int32)       # gtt8/yall8 scale
    # W fp8 scale: product scale 2^K uniform across head blocks
    wabs = np.array([np.abs(Wp[:, h * 2 * 128:(h + 1) * 2 * 128]).max()
                     for h in range(H)])
    K = int(np.floor(np.min(np.log2(224.0 / wabs) + seg)))
    swh = [2.0 ** (K - int(seg[h])) for h in range(H)]

    E4 = ml_dtypes.float8_e4m3
    wt8 = np.zeros((128, 6, 512), E4)
    for h in range(H):
        blk = Wp[:, h * 256:(h + 1) * 256] * swh[h]              # (512 o, 256 c)
        wt8[:, 2 * h:2 * h + 2, :] = (
            blk.T.reshape(2, 128, 512).transpose(1, 0, 2).astype(E4))

    i2k = (np.eye(128, dtype=f) * (2.0 ** K))
    fxp_d = (fxp * (2.0 ** -K)).astype(f)

    scales = np.zeros((128, 4), f)
    for h in range(H):
        scales[:, h] = 2.0 ** int(seg[h])

    bias_sb = np.stack([
        np.concatenate([theta_b[h].reshape(2, 128).T, phi_b[h].reshape(2, 128).T], axis=1)
        for h in range(H)]).astype(f)                   # (3, 128, 4)

    fxT_sb = fxp_d.T.reshape(4, 128, 512).transpose(1, 0, 2).reshape(128, 4 * 512).copy()
    cF_sb = cF.reshape(4, 128).T.copy()
    x_sb = x.reshape(B, 4, 128, T).transpose(0, 2, 1, 3)       # (B, 128, 4, T)

    common = {"fw": fw, "bias": bias_sb, "scales": scales, "wt8": wt8,
              "i2k": i2k, "fxT": fxT_sb, "cF": cF_sb}
    in_maps = []
    for c in range(NCORES):
        m = dict(common)
        xc = np.ascontiguousarray(x_sb[c * BPC:(c + 1) * BPC])
        m["x"] = xc
        m["x16"] = xc.astype(ml_dtypes.bfloat16)
        in_maps.append(m)
    return in_maps


def kernel(**inputs) -> np.ndarray:
    if "nc" not in _CACHE:
        _CACHE["nc"] = _build()
    nc = _CACHE["nc"]
    in_maps = _prep(inputs)
    res = bass_utils.run_bass_kernel_spmd(nc, in_maps, core_ids=list(range(NCORES)))
    out = np.empty((B, C, T), np.float32)
    for c in range(NCORES):
        out[c * BPC:(c + 1) * BPC] = res.results[c]["out"]
    return out


# revision 19
# speedup vs baseline: 1.4319x; 1.0037x over previous
"""Trainium2 Bass kernel for multi-head NonLocalBlock1D (B=16, C=512, T=1024, 3 heads).

Strategy:
  - Data-parallel over batch: 8 cores x 2 batches each, zero collectives.
  - Temporal dilated convs folded into the g/theta/phi 1x1 projections
    (host-side weight folding): proj(conv_h(x)) = sum_k (proj_w @ Ck) @ shift_k(x).
  - theta/phi/g projections run bf16 x bf16 (weights + a bf16 copy of x);
    scores + fx run in float32r; the residual uses a separate f32r x copy
    loaded late. Softmax normalization is applied to exp BEFORE the yraw matmul
    (weights in [0,1] scaled x128), which makes the yraw and W matmuls safe in
    fp8 e4m3 with perf_mode=DoubleRow (K=256 per matmul, 2x PE throughput):
      yraw = gtt8^T ydot8 (pairs over s-blocks), W = wt8^T yall8 (pairs over kc).
  - The residual x is added into the W psum via an identity matmul scaled 2^K
    (all fp8 scale factors are powers of two, folded into host-side weights,
    per-partition ACT scale tiles, and the descaled fx weights).
  - g biases ride through softmax (rows sum to 1) and are folded, with both
    BatchNorms and conv/proj bias terms, into W/fx weights + one final bias.
  - Heads processed in order [2, 0, 1] (head 2 has 1 tap -> short startup DMA),
    software-pipelined one head deep so the PE never waits on the
    exp->colsum->reciprocal->normalize chain.
"""
import numpy as np
import ml_dtypes

import concourse.bass as bass
import concourse.tile as tile
import concourse.mybir as mybir
from concourse import bacc, bass_utils
from contextlib import ExitStack

F32 = mybir.dt.float32
F32R = mybir.dt.float32r
F8 = mybir.dt.float8e4
BF16 = mybir.dt.bfloat16
AF = mybir.ActivationFunctionType
DR = mybir.MatmulPerfMode.DoubleRow

B, C, T, INTER, H, TL = 16, 512, 1024, 256, 3, 2
EPS = 1e-5
NCORES = 8
BPC = B // NCORES          # batches per core
XW = T + 4                 # padded x chunk width (+-2 zero pad)
HEAD_ORDER = (2, 0, 1)

_CACHE = {}


def _build():
    nc = bacc.Bacc("TRN2")
    x_d = nc.dram_tensor("x", (BPC, 128, 4, T), F32R, kind="ExternalInput")
    x16_d = nc.dram_tensor("x16", (BPC, 128, 4, T), BF16, kind="ExternalInput")
    fw_d = nc.dram_tensor("fw", (9, 128, 3 * 4 * INTER), BF16, kind="ExternalInput")
    bias_d = nc.dram_tensor("bias", (H, 128, 4), F32, kind="ExternalInput")
    scales_d = nc.dram_tensor("scales", (128, 4), F32, kind="ExternalInput")
    wt8_d = nc.dram_tensor("wt8", (128, 6, 512), F8, kind="ExternalInput")
    i2k_d = nc.dram_tensor("i2k", (128, 128), F32R, kind="ExternalInput")
    fxT_d = nc.dram_tensor("fxT", (128, 4 * 512), F32R, kind="ExternalInput")
    cF_d = nc.dram_tensor("cF", (128, 4), F32, kind="ExternalInput")
    out_d = nc.dram_tensor("out", (BPC, C, T), F32, kind="ExternalOutput")

    with tile.TileContext(nc) as tc, ExitStack() as ctx:
        def pool(name, bufs, **kw):
            return ctx.enter_context(tc.tile_pool(name=name, bufs=bufs, **kw))

        p_const = pool("const", 1)
        p_x = pool("xp", 2)
        p_fw = pool("fwp", 4)
        p_thph = pool("thph", 2)
        p_gt = pool("gtp", 2)
        p_exp = pool("expp", 9)
        p_yd = pool("ydp", 16)
        p_yall = pool("yallp", 2)
        p_misc = pool("miscp", 2)
        p_zr = pool("zrp", 8)
        p_out = pool("outp", 2)
        p_ps = pool("ps", 8, space="PSUM")

        zz = p_const.tile([128, 2], F32, tag="zz")
        nc.vector.memset(zz[:], 0.0)
        ones_f = p_const.tile([128, 1], F32, tag="ones_f")
        nc.vector.memset(ones_f[:], 0.0078125)     # 1/128, folds x128 into recip
        ones = p_const.tile([128, 1], F32R, tag="ones")
        nc.vector.tensor_copy(ones[:], ones_f[:])

        scales = p_const.tile([128, 4], F32, tag="scales")
        wt8 = p_const.tile([128, 6, 512], F8, tag="wt8")
        i2k = p_const.tile([128, 128], F32R, tag="i2k")
        fxt = p_const.tile([128, 4 * 512], F32R, tag="fxT")
        cft = p_const.tile([128, 4], F32, tag="cF")
        biases = [p_const.tile([128, 4], F32, tag=f"bias{h}", name=f"bias{h}")
                  for h in range(H)]

        def load_const_early():
            # scalar queue: tiny consts needed by the first ACTs
            nc.scalar.dma_start(scales[:], scales_d.ap()[:])
            for h in HEAD_ORDER:
                nc.scalar.dma_start(biases[h][:], bias_d.ap()[h])

        def load_const_late():
            # W/fx consts, needed only ~halfway through batch 0
            nc.scalar.dma_start(wt8[:], wt8_d.ap()[:])
            nc.scalar.dma_start(i2k[:], i2k_d.ap()[:])
            nc.scalar.dma_start(fxt[:], fxT_d.ap()[:])
            nc.scalar.dma_start(cft[:], cF_d.ap()[:])

        # ---------- x tiles (both batches), strided window DMAs ----------
        xts = []
        xfs = []
        for b in range(BPC):
            xt = p_x.tile([128, 4, XW], BF16, tag="x", name=f"x{b}")
            for cc in range(4):
                nc.vector.tensor_copy(xt[:, cc:cc + 1, 0:2], zz[:])
                nc.vector.tensor_copy(xt[:, cc:cc + 1, 2 + T:4 + T], zz[:])
            xts.append(xt)
            xf = p_x.tile([128, 4, T], F32R, tag="xf", name=f"xf{b}")
            xfs.append(xf)

        def load_x(b, windows):
            # one strided DMA per t-window covering all 4 c-chunks
            for lo, hi in windows:
                nc.sync.dma_start(xts[b][:, :, 2 + lo:2 + hi],
                                  x16_d.ap()[b][:, :, lo:hi])

        def load_xf(b):
            # full-precision x for the residual path, needed only at phase_W
            nc.sync.dma_start(xfs[b][:], x_d.ap()[b][:])

        def xs(b, cc, lo, width, dlt=0):
            base = 2 + lo + dlt
            return xts[b][:, cc:cc + 1, base:base + width]

        def xsf(b, cc, lo, width):
            return xfs[b][:, cc:cc + 1, lo:lo + width]

        # ---------- weight loads (gpsimd queue, one head ahead) ----------
        fwt = {}

        def load_fw(b, h, fine=False):
            nk = 3 if h < TL else 1
            for pj in range(3):             # 0=theta, 1=phi, 2=g
                t_ = p_fw.tile([128, 3 * 4 * INTER], BF16, tag="fw",
                               name=f"fw{b}{h}{pj}")
                nsplit = 2 if (fine and pj == 0) else 1
                w_ = nk * 4 * INTER // nsplit
                for ki in range(nsplit):
                    nc.gpsimd.dma_start(
                        t_[:, ki * w_:(ki + 1) * w_],
                        fw_d.ap()[h * 3 + pj][:, ki * w_:(ki + 1) * w_])
                fwt[(h, pj)] = t_

        # ================= per-head phases =================
        def phase_A(b, h, hook=None):
            """theta/phi ([i,t] f32r) + gT (-> gtt8 fp8), t-half n outermost."""
            taps = [-(h + 1), 0, h + 1] if h < TL else [0]
            nk = len(taps)
            tht = p_thph.tile([128, 2 * T], F32R, tag="th", name=f"th{b}{h}")
            pht = p_thph.tile([128, 2 * T], F32R, tag="ph", name=f"ph{b}{h}")
            gtt8 = p_gt.tile([128, 8, INTER], F8, tag="gt", name=f"gt{b}{h}")
            for n in range(2):
                for pj, dst in ((0, tht), (1, pht)):
                    for it in range(2):
                        ps = p_ps.tile([128, 512], F32, tag="ps")
                        cnt = 0
                        for ki, dlt in enumerate(taps):
                            for cc in range(4):
                                lhs = fwt[(h, pj)][:, (ki * 4 + cc) * INTER + it * 128:
                                                   (ki * 4 + cc) * INTER + (it + 1) * 128]
                                nc.tensor.matmul(
                                    ps[:], lhs, xs(b, cc, n * 512, 512, dlt),
                                    start=(cnt == 0), stop=(cnt == nk * 4 - 1))
                                cnt += 1
                        nc.scalar.activation(
                            dst[:, it * T + n * 512:it * T + (n + 1) * 512], ps[:],
                            AF.Identity,
                            bias=biases[h][:, pj * 2 + it:pj * 2 + it + 1])
                for sb in range(4 * n, 4 * n + 4):
                    ps = p_ps.tile([128, 512], F32, tag="ps")
                    cnt = 0
                    for ki, dlt in enumerate(taps):
                        for cc in range(4):
                            nc.tensor.matmul(
                                ps[:, 0:INTER],
                                xs(b, cc, sb * 128, 128, dlt),
                                fwt[(h, 2)][:, (ki * 4 + cc) * INTER:(ki * 4 + cc + 1) * INTER],
                                start=(cnt == 0), stop=(cnt == nk * 4 - 1))
                            cnt += 1
                    nc.scalar.activation(gtt8[:, sb:sb + 1, :], ps[:, 0:INTER],
                                         AF.Identity, scale=scales[:, h:h + 1])
                if n == 0 and hook is not None:
                    hook()
            return tht, pht, gtt8

        def phase_B(b, h, tht, pht):
            """scores -> exp -> colsum -> recip -> normalized fp8 weights."""
            yd_all = []
            for n in range(2):
                cst = p_ps.tile([128, 512], F32, tag="ps")
                exs = []
                for sb in range(8):
                    scp = p_ps.tile([128, 512], F32, tag="ps")
                    for ic in range(2):
                        nc.tensor.matmul(
                            scp[:],
                            pht[:, ic * T + sb * 128:ic * T + (sb + 1) * 128],
                            tht[:, ic * T + n * 512:ic * T + (n + 1) * 512],
                            start=(ic == 0), stop=(ic == 1))
                    ex = p_exp.tile([128, 512], F32R, tag="exp")
                    nc.scalar.activation(ex[:], scp[:], AF.Exp)
                    exs.append(ex)
                    nc.tensor.matmul(cst[0:1, :], ones[:], ex[:],
                                     start=(sb == 0), stop=(sb == 7))
                rcs = p_misc.tile([128, 512], F32, tag="cs", bufs=2, name="rcs")
                nc.vector.reciprocal_approx_fast(rcs[0:1, :], cst[0:1, :])
                rbc = p_misc.tile([128, 512], F32, tag="rbc", bufs=2)
                nc.gpsimd.partition_broadcast(rbc[:], rcs[0:1, :])
                yds = []
                for p in range(4):
                    yd = p_yd.tile([128, 2, 512], F8, tag="yd")
                    for j in range(2):
                        nc.vector.tensor_mul(yd[:, j:j + 1, :], exs[2 * p + j][:],
                                             rbc[:])
                    yds.append(yd)
                yd_all.append(yds)
            return yd_all

        def phase_C(b, h, gtt8, yd_all, yall8):
            """yraw via fp8 DoubleRow -> yall8."""
            for n in range(2):
                yds = yd_all[n]
                for ic in range(2):
                    yr = p_ps.tile([128, 512], F32, tag="ps")
                    for p in range(4):
                        nc.tensor.matmul(
                            yr[:],
                            gtt8[:, 2 * p:2 * p + 2, ic * 128:(ic + 1) * 128],
                            yds[p][:, :, :],
                            start=(p == 0), stop=(p == 3), perf_mode=DR)
                    kc = h * 2 + ic
                    nc.scalar.activation(
                        yall8[:, kc:kc + 1, n * 512:(n + 1) * 512], yr[:],
                        AF.Identity, scale=0.0078125)

        def phase_W(b, yall8):
            """W (fp8 DoubleRow) + scaled-identity residual -> zr tiles."""
            zrs = []
            for n in range(2):
                zrt = []
                for oc in range(4):
                    ps = p_ps.tile([128, 512], F32, tag="ps")
                    nc.tensor.matmul(ps[:], i2k[:], xsf(b, oc, n * 512, 512),
                                     start=True, stop=False,
                                     skip_group_check=True)
                    for q in range(3):
                        nc.tensor.matmul(
                            ps[:],
                            wt8[:, 2 * q:2 * q + 2, oc * 128:(oc + 1) * 128],
                            yall8[:, 2 * q:2 * q + 2, n * 512:(n + 1) * 512],
                            start=False, stop=(q == 2), perf_mode=DR,
                            skip_group_check=True)
                    zr = p_zr.tile([128, 512], F32R, tag="zr")
                    nc.vector.tensor_copy(zr[:], ps[:])
                    zrt.append(zr)
                zrs.append(zrt)
            return zrs

        OUT_Q = (nc.sync, nc.gpsimd, nc.scalar, nc.sync)

        def phase_fx(b, zrs):
            for n in range(2):
                zrt = zrs[n]
                for mo in range(4):
                    ps = p_ps.tile([128, 512], F32, tag="ps")
                    for kc in range(4):
                        nc.tensor.matmul(
                            ps[:],
                            fxt[:, kc * 512 + mo * 128:kc * 512 + (mo + 1) * 128],
                            zrt[kc][:],
                            start=(kc == 0), stop=(kc == 3))
                    ot = p_out.tile([128, 512], F32, tag="o")
                    nc.scalar.activation(ot[:], ps[:], AF.Identity,
                                         bias=cft[:, mo:mo + 1])
                    OUT_Q[mo].dma_start(
                        out_d.ap()[b, mo * 128:(mo + 1) * 128,
                                   n * 512:(n + 1) * 512],
                        ot[:])

        # ================= schedule =================
        # Per-queue DMA completion semaphores are monotonic: a consumer waits
        # for every earlier-emitted DMA on that queue. So emission order on
        # each queue strictly follows consumption order, and batch-1 loads
        # are deferred past batch-0's hot startup window.
        load_seq = [(b, h) for b in range(BPC) for h in HEAD_ORDER]
        load_fw(*load_seq[0], fine=True)
        load_x(0, [(0, 260), (260, 516)])
        load_const_early()
        next_load = 1

        def emit_next_load():
            nonlocal next_load
            if next_load < len(load_seq):
                load_fw(*load_seq[next_load])
                next_load += 1

        pend = None        # (b, h, gtt8, yd_all, yall8) awaiting phase_C
        done_A = {}
        for b in range(BPC):
            yall8 = p_yall.tile([128, 6, T], F8, tag="yall", name=f"yall{b}")
            for hi, h in enumerate(HEAD_ORDER):
                if (b, h) in done_A:
                    tht, pht, gtt8 = done_A.pop((b, h))
                else:
                    hook = (lambda: load_x(0, [(516, 772), (772, 1024)])) \
                        if (b == 0 and hi == 0) else None
                    tht, pht, gtt8 = phase_A(b, h, hook)
                    emit_next_load()
                if hi == 1:
                    load_xf(b)
                if b == 0 and hi == 1:
                    load_const_late()
                if b == 0 and hi == 2:
                    load_x(1, [(0, 516), (516, 1024)])
                if pend is not None:
                    phase_C(*pend)
                    pend = None
                yd_all = phase_B(b, h, tht, pht)
                pend = (b, h, gtt8, yd_all, yall8)
            phase_C(*pend)
            pend = None
            zrs = phase_W(b, yall8)
            if b + 1 < BPC:           # cover the W->zr->fx stall with next A
                done_A[(b + 1, HEAD_ORDER[0])] = phase_A(b + 1, HEAD_ORDER[0])
                emit_next_load()
            phase_fx(b, zrs)

    nc.compile()
    return nc


def _prep(inputs):
    f = np.float32
    x = np.asarray(inputs["x"], f)
    tconv_w = np.asarray(inputs["tconv_w"], f)
    g_w = np.asarray(inputs["g_w"], f)
    g_b = np.asarray(inputs["g_b"], f)
    theta_w = np.asarray(inputs["theta_w"], f)
    theta_b = np.asarray(inputs["theta_b"], f)
    phi_w = np.asarray(inputs["phi_w"], f)
    phi_b = np.asarray(inputs["phi_b"], f)
    W_w = np.asarray(inputs["W_w"], f)
    W_b = np.asarray(inputs["W_b"], f)

    s1 = np.asarray(inputs["bn1_gamma"], f) / np.sqrt(np.asarray(inputs["bn1_var"], f) + EPS)
    s2 = np.asarray(inputs["bn2_gamma"], f) / np.sqrt(np.asarray(inputs["bn2_var"], f) + EPS)
    fx_w = np.asarray(inputs["fx_w"], f)

    # fold g biases (softmax rows sum to 1) + BN1 into W / cz
    g_ball = g_b.reshape(H * INTER)
    Wp = (W_w * s1[:, None]).astype(f)
    cz = (s1 * (W_w @ g_ball + W_b - np.asarray(inputs["bn1_mean"], f))
          + np.asarray(inputs["bn1_beta"], f)).astype(f)
    fxp = (fx_w * s2[:, None]).astype(f)
    cF = (s2 * (fx_w @ cz + np.asarray(inputs["fx_b"], f) - np.asarray(inputs["bn2_mean"], f))
          + np.asarray(inputs["bn2_beta"], f)).astype(f)

    # folded projection weights (bf16), [c, i] layout per (h, proj, tap)
    fw = np.zeros((9, 128, 3 * 4 * INTER), ml_dtypes.bfloat16)
    folds_g = []
    for h in range(H):
        for pj, pw in enumerate((theta_w, phi_w, g_w)):
            if h < TL:
                folds = [(pw[h] @ tconv_w[h, :, 0, k, :]).T for k in range(3)]
            else:
                folds = [pw[h].T]
            if pj == 2:
                folds_g.append(folds)
            arr = np.stack(folds)                      # (nk, 512, 256)
            nk = arr.shape[0]
            arr = arr.reshape(nk, 4, 128, INTER).transpose(2, 0, 1, 3)
            fw[h * 3 + pj, :, :nk * 4 * INTER] = arr.reshape(128, nk * 4 * INTER)

    # exact per-head |g| bound (|y| <= max|g| since softmax rows are convex)
    gmax = np.zeros(H, f)
    xp = np.pad(x, ((0, 0), (0, 0), (2, 2)))
    for h in range(H):
        taps = [-(h + 1), 0, h + 1] if h < TL else [0]
        gh = np.zeros((B, INTER, T), np.float32)
        for ki, dlt in enumerate(taps):
            gh += np.einsum("ci,bct->bit", folds_g[h][ki],
                            xp[:, :, 2 + dlt:2 + dlt + T].astype(np.float32))
        gmax[h] = np.abs(gh).max()

    seg = np.floor(np.log2(224.0 / gmax)).astype(np.int32)       # gtt8/yall8 scale
    # W fp8 scale: product scale 2^K uniform across head blocks
    wabs = np.array([np.abs(Wp[:, h * 2 * 128:(h + 1) * 2 * 128]).max()
                     for h in range(H)])
    K = int(np.floor(np.min(np.log2(224.0 / wabs) + seg)))
    swh = [2.0 ** (K - int(seg[h])) for h in range(H)]

    E4 = ml_dtypes.float8_e4m3
    wt8 = np.zeros((128, 6, 512), E4)
    for h in range(H):
        blk = Wp[:, h * 256:(h + 1) * 256] * swh[h]              # (512 o, 256 c)
        wt8[:, 2 * h:2 * h + 2, :] = (
            blk.T.reshape(2, 128, 512).transpose(1, 0, 2).astype(E4))

    i2k = (np.eye(128, dtype=f) * (2.0 ** K))
    fxp_d = (fxp * (2.0 ** -K)).astype(f)

    scales = np.zeros((128, 4), f)
    for h in range(H):
        scales[:, h] = 2.0 ** int(seg[h])

    bias_sb = np.stack([
        np.concatenate([theta_b[h].reshape(2, 128).T, phi_b[h].reshape(2, 128).T], axis=1)
        for h in range(H)]).astype(f)                   # (3, 128, 4)

    fxT_sb = fxp_d.T.reshape(4, 128, 512).transpose(1, 0, 2).reshape(128, 4 * 512).copy()
    cF_sb = cF.reshape(4, 128).T.copy()
    x_sb = x.reshape(B, 4, 128, T).transpose(0, 2, 1, 3)       # (B, 128, 4, T)

    common = {"fw": fw, "bias": bias_sb, "scales": scales, "wt8": wt8,
              "i2k": i2k, "fxT": fxT_sb, "cF": cF_sb}
    in_maps = []
    for c in range(NCORES):
        m = dict(common)
        xc = np.ascontiguousarray(x_sb[c * BPC:(c + 1) * BPC])
        m["x"] = xc
        m["x16"] = xc.astype(ml_dtypes.bfloat16)
        in_maps.append(m)
    return in_maps


def kernel(**inputs) -> np.ndarray:
    if "nc" not in _CACHE:
        _CACHE["nc"] = _build()
    nc = _CACHE["nc"]
    in_maps = _prep(inputs)
    res = bass_utils.run_bass_kernel_spmd(nc, in_maps, core_ids=list(range(NCORES)))
    out = np.empty((B, C, T), np.float32)
    for c in range(NCORES):
        out[c * BPC:(c + 1) * BPC] = res.results[c]["out"]
    return out


# revision 25
# speedup vs baseline: 1.5272x; 1.0666x over previous
"""Trainium2 Bass kernel for multi-head NonLocalBlock1D (B=16, C=512, T=1024, 3 heads).

Strategy:
  - Data-parallel over batch: 8 cores x 2 batches each, zero collectives.
  - Temporal dilated convs folded into the g/theta/phi 1x1 projections
    (host-side weight folding): proj(conv_h(x)) = sum_k (proj_w @ Ck) @ shift_k(x).
  - theta/phi/g projections run bf16 x bf16 (weights + a bf16 copy of x);
    fx runs float32r; the residual uses a separate f32r x copy loaded late.
    Scores run fp8 e4m3 DoubleRow (theta/phi stored x64, one K=256 matmul per
    s-block with the stationary operand reused across both t-chunks; exp
    descales by 2^-12). Softmax normalization is applied to exp BEFORE the
    yraw matmul (weights in [0,1] scaled x128), which makes the yraw and W
    matmuls safe in fp8 e4m3 DoubleRow as well:
      yraw = gtt8^T ydot8 (pairs over s-blocks), W = wt8^T yall8 (pairs over kc).
  - The residual x is added into the W psum via an identity matmul scaled 2^K
    (all fp8 scale factors are powers of two, folded into host-side weights,
    per-partition ACT scale tiles, and the descaled fx weights).
  - g biases ride through softmax (rows sum to 1) and are folded, with both
    BatchNorms and conv/proj bias terms, into W/fx weights + one final bias.
  - Heads processed in order [2, 0, 1] (head 2 has 1 tap -> short startup DMA),
    software-pipelined one head deep so the PE never waits on the
    exp->colsum->reciprocal->normalize chain.
"""
import numpy as np
import ml_dtypes

import concourse.bass as bass
import concourse.tile as tile
import concourse.mybir as mybir
from concourse import bacc, bass_utils
from contextlib import ExitStack

F32 = mybir.dt.float32
F32R = mybir.dt.float32r
F8 = mybir.dt.float8e4
BF16 = mybir.dt.bfloat16
AF = mybir.ActivationFunctionType
DR = mybir.MatmulPerfMode.DoubleRow

B, C, T, INTER, H, TL = 16, 512, 1024, 256, 3, 2
EPS = 1e-5
NCORES = 8
BPC = B // NCORES          # batches per core
XW = T + 4                 # padded x chunk width (+-2 zero pad)
HEAD_ORDER = (2, 0, 1)

_CACHE = {}


def _build():
    nc = bacc.Bacc("TRN2")
    x_d = nc.dram_tensor("x", (BPC, 128, 4, T), F32R, kind="ExternalInput")
    x16_d = nc.dram_tensor("x16", (BPC, 128, 4, T), BF16, kind="ExternalInput")
    fw_d = nc.dram_tensor("fw", (9, 128, 3 * 4 * INTER), BF16, kind="ExternalInput")
    bias_d = nc.dram_tensor("bias", (H, 128, 4), F32, kind="ExternalInput")
    scales_d = nc.dram_tensor("scales", (128, 4), F32, kind="ExternalInput")
    wt8_d = nc.dram_tensor("wt8", (128, 6, 512), F8, kind="ExternalInput")
    i2k_d = nc.dram_tensor("i2k", (128, 128), F32R, kind="ExternalInput")
    fxT_d = nc.dram_tensor("fxT", (128, 4 * 512), F32R, kind="ExternalInput")
    cF_d = nc.dram_tensor("cF", (128, 4), F32, kind="ExternalInput")
    out_d = nc.dram_tensor("out", (BPC, C, T), F32, kind="ExternalOutput")

    with tile.TileContext(nc) as tc, ExitStack() as ctx:
        def pool(name, bufs, **kw):
            return ctx.enter_context(tc.tile_pool(name=name, bufs=bufs, **kw))

        p_const = pool("const", 1)
        p_x = pool("xp", 2)
        p_fw = pool("fwp", 4)
        p_thph = pool("thph", 2)
        p_gt = pool("gtp", 2)
        p_exp = pool("expp", 18)
        p_yd = pool("ydp", 16)
        p_yall = pool("yallp", 2)
        p_misc = pool("miscp", 2)
        p_zr = pool("zrp", 8)
        p_out = pool("outp", 2)
        p_ps = pool("ps", 8, space="PSUM")

        zz = p_const.tile([128, 2], F32, tag="zz")
        nc.vector.memset(zz[:], 0.0)
        ones_f = p_const.tile([128, 1], F32, tag="ones_f")
        nc.vector.memset(ones_f[:], 0.0078125)     # 1/128, folds x128 into recip
        ones = p_const.tile([128, 1], F32R, tag="ones")
        nc.vector.tensor_copy(ones[:], ones_f[:])

        scales = p_const.tile([128, 4], F32, tag="scales")
        wt8 = p_const.tile([128, 6, 512], F8, tag="wt8")
        i2k = p_const.tile([128, 128], F32R, tag="i2k")
        fxt = p_const.tile([128, 4 * 512], F32R, tag="fxT")
        cft = p_const.tile([128, 4], F32, tag="cF")
        biases = [p_const.tile([128, 4], F32, tag=f"bias{h}", name=f"bias{h}")
                  for h in range(H)]

        def load_const_early():
            # scalar queue: tiny consts needed by the first ACTs
            nc.scalar.dma_start(scales[:], scales_d.ap()[:])
            for h in HEAD_ORDER:
                nc.scalar.dma_start(biases[h][:], bias_d.ap()[h])

        def load_const_late():
            # W/fx consts, needed only ~halfway through batch 0
            nc.scalar.dma_start(wt8[:], wt8_d.ap()[:])
            nc.scalar.dma_start(i2k[:], i2k_d.ap()[:])
            nc.scalar.dma_start(fxt[:], fxT_d.ap()[:])
            nc.scalar.dma_start(cft[:], cF_d.ap()[:])

        # ---------- x tiles (both batches), strided window DMAs ----------
        xts = []
        xfs = []
        for b in range(BPC):
            xt = p_x.tile([128, 4, XW], BF16, tag="x", name=f"x{b}")
            for cc in range(4):
                nc.vector.tensor_copy(xt[:, cc:cc + 1, 0:2], zz[:])
                nc.vector.tensor_copy(xt[:, cc:cc + 1, 2 + T:4 + T], zz[:])
            xts.append(xt)
            xf = p_x.tile([128, 4, T], F32R, tag="xf", name=f"xf{b}")
            xfs.append(xf)

        def load_x(b, windows):
            # one strided DMA per t-window covering all 4 c-chunks
            for lo, hi in windows:
                nc.sync.dma_start(xts[b][:, :, 2 + lo:2 + hi],
                                  x16_d.ap()[b][:, :, lo:hi])

        def load_xf(b):
            # full-precision x for the residual path, needed only at phase_W
            nc.sync.dma_start(xfs[b][:], x_d.ap()[b][:])

        def xs(b, cc, lo, width, dlt=0):
            base = 2 + lo + dlt
            return xts[b][:, cc:cc + 1, base:base + width]

        def xsf(b, cc, lo, width):
            return xfs[b][:, cc:cc + 1, lo:lo + width]

        # ---------- weight loads (gpsimd queue, one head ahead) ----------
        fwt = {}

        def load_fw_pj(b, h, pj, fine=False):
            nk = 3 if h < TL else 1
            t_ = p_fw.tile([128, 3 * 4 * INTER], BF16, tag="fw",
                           name=f"fw{b}{h}{pj}")
            nsplit = 2 if (fine and pj == 0) else 1
            w_ = nk * 4 * INTER // nsplit
            for ki in range(nsplit):
                nc.gpsimd.dma_start(
                    t_[:, ki * w_:(ki + 1) * w_],
                    fw_d.ap()[h * 3 + pj][:, ki * w_:(ki + 1) * w_])
            fwt[(h, pj)] = t_

        def load_fw(b, h, fine=False):
            for pj in range(3):             # 0=theta, 1=phi, 2=g
                load_fw_pj(b, h, pj, fine)

        # ================= per-head phases =================
        def phase_A(b, h, hook=None, loader=None):
            """theta/phi (fp8 x64, [i-pair, t]) + gT (-> gtt8 fp8)."""
            taps = [-(h + 1), 0, h + 1] if h < TL else [0]
            nk = len(taps)
            tht = p_thph.tile([128, 2, T], F8, tag="th", name=f"th{b}{h}")
            pht = p_thph.tile([128, 2, T], F8, tag="ph", name=f"ph{b}{h}")
            gtt8 = p_gt.tile([128, 8, INTER], F8, tag="gt", name=f"gt{b}{h}")
            for n in range(2):
                for pj, dst in ((0, tht), (1, pht)):
                    if n == 0 and loader is not None:
                        loader(pj)
                    for it in range(2):
                        ps = p_ps.tile([128, 512], F32, tag="ps")
                        cnt = 0
                        for ki, dlt in enumerate(taps):
                            for cc in range(4):
                                lhs = fwt[(h, pj)][:, (ki * 4 + cc) * INTER + it * 128:
                                                   (ki * 4 + cc) * INTER + (it + 1) * 128]
                                nc.tensor.matmul(
                                    ps[:], lhs, xs(b, cc, n * 512, 512, dlt),
                                    start=(cnt == 0), stop=(cnt == nk * 4 - 1))
                                cnt += 1
                        nc.scalar.activation(
                            dst[:, it:it + 1, n * 512:(n + 1) * 512], ps[:],
                            AF.Identity, scale=64.0,
                            bias=biases[h][:, pj * 2 + it:pj * 2 + it + 1])
                if n == 0 and loader is not None:
                    loader(2)
                for sb in range(4 * n, 4 * n + 4):
                    ps = p_ps.tile([128, 512], F32, tag="ps")
                    cnt = 0
                    for ki, dlt in enumerate(taps):
                        for cc in range(4):
                            nc.tensor.matmul(
                                ps[:, 0:INTER],
                                xs(b, cc, sb * 128, 128, dlt),
                                fwt[(h, 2)][:, (ki * 4 + cc) * INTER:(ki * 4 + cc + 1) * INTER],
                                start=(cnt == 0), stop=(cnt == nk * 4 - 1))
                            cnt += 1
                    nc.scalar.activation(gtt8[:, sb:sb + 1, :], ps[:, 0:INTER],
                                         AF.Identity, scale=scales[:, h:h + 1])
                if n == 0 and hook is not None:
                    hook()
            return tht, pht, gtt8

        def phase_B(b, h, tht, pht):
            """scores -> exp -> colsum -> recip -> normalized fp8 weights."""
            csts = [p_ps.tile([128, 512], F32, tag="ps", name=f"cst{n}")
                    for n in range(2)]
            exs = [[None] * 8, [None] * 8]
            for sb in range(8):          # fp8 stationary reused for both n
                for n in range(2):
                    scp = p_ps.tile([128, 512], F32, tag="ps")
                    nc.tensor.matmul(
                        scp[:],
                        pht[:, :, sb * 128:(sb + 1) * 128],
                        tht[:, :, n * 512:(n + 1) * 512],
                        start=True, stop=True, perf_mode=DR)
                    ex = p_exp.tile([128, 512], F32R, tag="exp")
                    nc.scalar.activation(ex[:], scp[:], AF.Exp, scale=2.0 ** -12)
                    exs[n][sb] = ex
                    nc.tensor.matmul(csts[n][0:1, :], ones[:], ex[:],
                                     start=(sb == 0), stop=(sb == 7))
            yd_all = []
            for n in range(2):
                rcs = p_misc.tile([128, 512], F32, tag="cs", bufs=2, name="rcs")
                nc.vector.reciprocal_approx_fast(rcs[0:1, :], csts[n][0:1, :])
                rbc = p_misc.tile([128, 512], F32, tag="rbc", bufs=2)
                nc.gpsimd.partition_broadcast(rbc[:], rcs[0:1, :])
                yds = []
                for p in range(4):
                    yd = p_yd.tile([128, 2, 512], F8, tag="yd")
                    for j in range(2):
                        nc.vector.tensor_mul(yd[:, j:j + 1, :], exs[n][2 * p + j][:],
                                             rbc[:])
                    yds.append(yd)
                yd_all.append(yds)
            return yd_all

        def phase_C(b, h, gtt8, yd_all, yall8):
            """yraw via fp8 DoubleRow -> yall8."""
            for n in range(2):
                yds = yd_all[n]
                for ic in range(2):
                    yr = p_ps.tile([128, 512], F32, tag="ps")
                    for p in range(4):
                        nc.tensor.matmul(
                            yr[:],
                            gtt8[:, 2 * p:2 * p + 2, ic * 128:(ic + 1) * 128],
                            yds[p][:, :, :],
                            start=(p == 0), stop=(p == 3), perf_mode=DR)
                    kc = h * 2 + ic
                    nc.scalar.activation(
                        yall8[:, kc:kc + 1, n * 512:(n + 1) * 512], yr[:],
                        AF.Identity, scale=0.0078125)

        def phase_W(b, yall8):
            """W (fp8 DoubleRow) + scaled-identity residual -> zr tiles."""
            zrs = []
            for n in range(2):
                zrt = []
                for oc in range(4):
                    ps = p_ps.tile([128, 512], F32, tag="ps")
                    nc.tensor.matmul(ps[:], i2k[:], xsf(b, oc, n * 512, 512),
                                     start=True, stop=False,
                                     skip_group_check=True)
                    for q in (0, 2, 1):     # head-1's pair last: it finishes last
                        nc.tensor.matmul(
                            ps[:],
                            wt8[:, 2 * q:2 * q + 2, oc * 128:(oc + 1) * 128],
                            yall8[:, 2 * q:2 * q + 2, n * 512:(n + 1) * 512],
                            start=False, stop=(q == 1), perf_mode=DR,
                            skip_group_check=True)
                    zr = p_zr.tile([128, 512], F32R, tag="zr")
                    nc.vector.tensor_copy(zr[:], ps[:])
                    zrt.append(zr)
                zrs.append(zrt)
            return zrs

        OUT_Q = (nc.sync, nc.gpsimd, nc.scalar, nc.sync)

        def phase_fx(b, zrs):
            for n in range(2):
                zrt = zrs[n]
                for mo in range(4):
                    ps = p_ps.tile([128, 512], F32, tag="ps")
                    for kc in range(4):
                        nc.tensor.matmul(
                            ps[:],
                            fxt[:, kc * 512 + mo * 128:kc * 512 + (mo + 1) * 128],
                            zrt[kc][:],
                            start=(kc == 0), stop=(kc == 3))
                    ot = p_out.tile([128, 512], F32, tag="o")
                    nc.scalar.activation(ot[:], ps[:], AF.Identity,
                                         bias=cft[:, mo:mo + 1])
                    OUT_Q[mo].dma_start(
                        out_d.ap()[b, mo * 128:(mo + 1) * 128,
                                   n * 512:(n + 1) * 512],
                        ot[:])

        # ================= schedule =================
        # Per-queue DMA completion semaphores are monotonic: a consumer waits
        # for every earlier-emitted DMA on that queue. So emission order on
        # each queue strictly follows consumption order, and batch-1 loads
        # are deferred past batch-0's hot startup window.
        load_seq = [(b, h) for b in range(BPC) for h in HEAD_ORDER]
        load_x(0, [(0, 260), (260, 516)])
        load_const_early()
        next_load = 1

        def emit_next_load():
            nonlocal next_load
            if next_load < len(load_seq):
                load_fw(*load_seq[next_load])
                next_load += 1

        pend = None        # (b, h, gtt8, yd_all, yall8) awaiting phase_C
        done_A = {}
        for b in range(BPC):
            yall8 = p_yall.tile([128, 6, T], F8, tag="yall", name=f"yall{b}")
            for hi, h in enumerate(HEAD_ORDER):
                if (b, h) in done_A:
                    tht, pht, gtt8 = done_A.pop((b, h))
                else:
                    first = (b == 0 and hi == 0)
                    hook = (lambda: load_x(0, [(516, 772), (772, 1024)])) \
                        if first else None
                    loader = (lambda pj: load_fw_pj(0, HEAD_ORDER[0], pj,
                                                    fine=True)) if first else None
                    tht, pht, gtt8 = phase_A(b, h, hook, loader)
                    emit_next_load()
                if hi == 1:
                    load_xf(b)
                if b == 0 and hi == 1:
                    load_const_late()
                if b == 0 and hi == 2:
                    load_x(1, [(0, 516), (516, 1024)])
                if pend is not None:
                    phase_C(*pend)
                    pend = None
                yd_all = phase_B(b, h, tht, pht)
                pend = (b, h, gtt8, yd_all, yall8)
            if b + 1 < BPC:           # cover the last head's normalize chain
                done_A[(b + 1, HEAD_ORDER[0])] = phase_A(b + 1, HEAD_ORDER[0])
                emit_next_load()
            phase_C(*pend)
            pend = None
            zrs = phase_W(b, yall8)
            phase_fx(b, zrs)

    nc.compile()
    return nc


def _prep(inputs):
    f = np.float32
    x = np.asarray(inputs["x"], f)
    tconv_w = np.asarray(inputs["tconv_w"], f)
    g_w = np.asarray(inputs["g_w"], f)
    g_b = np.asarray(inputs["g_b"], f)
    theta_w = np.asarray(inputs["theta_w"], f)
    theta_b = np.asarray(inputs["theta_b"], f)
    phi_w = np.asarray(inputs["phi_w"], f)
    phi_b = np.asarray(inputs["phi_b"], f)
    W_w = np.asarray(inputs["W_w"], f)
    W_b = np.asarray(inputs["W_b"], f)

    s1 = np.asarray(inputs["bn1_gamma"], f) / np.sqrt(np.asarray(inputs["bn1_var"], f) + EPS)
    s2 = np.asarray(inputs["bn2_gamma"], f) / np.sqrt(np.asarray(inputs["bn2_var"], f) + EPS)
    fx_w = np.asarray(inputs["fx_w"], f)

    # fold g biases (softmax rows sum to 1) + BN1 into W / cz
    g_ball = g_b.reshape(H * INTER)
    Wp = (W_w * s1[:, None]).astype(f)
    cz = (s1 * (W_w @ g_ball + W_b - np.asarray(inputs["bn1_mean"], f))
          + np.asarray(inputs["bn1_beta"], f)).astype(f)
    fxp = (fx_w * s2[:, None]).astype(f)
    cF = (s2 * (fx_w @ cz + np.asarray(inputs["fx_b"], f) - np.asarray(inputs["bn2_mean"], f))
          + np.asarray(inputs["bn2_beta"], f)).astype(f)

    # folded projection weights (bf16), [c, i] layout per (h, proj, tap)
    fw = np.zeros((9, 128, 3 * 4 * INTER), ml_dtypes.bfloat16)
    folds_g = []
    for h in range(H):
        for pj, pw in enumerate((theta_w, phi_w, g_w)):
            if h < TL:
                folds = [(pw[h] @ tconv_w[h, :, 0, k, :]).T for k in range(3)]
            else:
                folds = [pw[h].T]
            if pj == 2:
                folds_g.append(folds)
            arr = np.stack(folds)                      # (nk, 512, 256)
            nk = arr.shape[0]
            arr = arr.reshape(nk, 4, 128, INTER).transpose(2, 0, 1, 3)
            fw[h * 3 + pj, :, :nk * 4 * INTER] = arr.reshape(128, nk * 4 * INTER)

    # exact per-head |g| bound (|y| <= max|g| since softmax rows are convex)
    gmax = np.zeros(H, f)
    xp = np.pad(x, ((0, 0), (0, 0), (2, 2)))
    for h in range(H):
        taps = [-(h + 1), 0, h + 1] if h < TL else [0]
        gh = np.zeros((B, INTER, T), np.float32)
        for ki, dlt in enumerate(taps):
            gh += np.einsum("ci,bct->bit", folds_g[h][ki],
                            xp[:, :, 2 + dlt:2 + dlt + T].astype(np.float32))
        gmax[h] = np.abs(gh).max()

    seg = np.floor(np.log2(224.0 / gmax)).astype(np.int32)       # gtt8/yall8 scale
    # W fp8 scale: product scale 2^K uniform across head blocks
    wabs = np.array([np.abs(Wp[:, h * 2 * 128:(h + 1) * 2 * 128]).max()
                     for h in range(H)])
    K = int(np.floor(np.min(np.log2(224.0 / wabs) + seg)))
    swh = [2.0 ** (K - int(seg[h])) for h in range(H)]

    E4 = ml_dtypes.float8_e4m3
    wt8 = np.zeros((128, 6, 512), E4)
    for h in range(H):
        blk = Wp[:, h * 256:(h + 1) * 256] * swh[h]              # (512 o, 256 c)
        wt8[:, 2 * h:2 * h + 2, :] = (
            blk.T.reshape(2, 128, 512).transpose(1, 0, 2).astype(E4))

    i2k = (np.eye(128, dtype=f) * (2.0 ** K))
    fxp_d = (fxp * (2.0 ** -K)).astype(f)

    scales = np.zeros((128, 4), f)
    for h in range(H):
        scales[:, h] = 2.0 ** int(seg[h])

    bias_sb = np.stack([
        np.concatenate([theta_b[h].reshape(2, 128).T, phi_b[h].reshape(2, 128).T], axis=1)
        for h in range(H)]).astype(f) * 64.0            # (3, 128, 4), rides ACT x64

    fxT_sb = fxp_d.T.reshape(4, 128, 512).transpose(1, 0, 2).reshape(128, 4 * 512).copy()
    cF_sb = cF.reshape(4, 128).T.copy()
    x_sb = x.reshape(B, 4, 128, T).transpose(0, 2, 1, 3)       # (B, 128, 4, T)

    common = {"fw": fw, "bias": bias_sb, "scales": scales, "wt8": wt8,
              "i2k": i2k, "fxT": fxT_sb, "cF": cF_sb}
    in_maps = []
    for c in range(NCORES):
        m = dict(common)
        xc = np.ascontiguousarray(x_sb[c * BPC:(c + 1) * BPC])
        m["x"] = xc
        m["x16"] = xc.astype(ml_dtypes.bfloat16)
        in_maps.append(m)
    return in_maps


def kernel(**inputs) -> np.ndarray:
    if "nc" not in _CACHE:
        _CACHE["nc"] = _build()
    nc = _CACHE["nc"]
    in_maps = _prep(inputs)
    res = bass_utils.run_bass_kernel_spmd(nc, in_maps, core_ids=list(range(NCORES)))
    out = np.empty((B, C, T), np.float32)
    for c in range(NCORES):
        out[c * BPC:(c + 1) * BPC] = res.results[c]["out"]
    return out
